# revision 44
# baseline (speedup 1.0000x reference)
"""Trainium2 Bass kernel for the sparse submanifold 3D CNN (nn_Net_38963943309313).

Network: 7 blocks of 2 submanifold 3x3x3 convs on a 64^3 grid, 2x2x2 sparse
max-pools between blocks, channels 3->64->...->256, output [1,1,1,1,256].

Strategy (8 NeuronCores):
 - Shard z-slabs across cores for levels 0-2 (grids 64/32/16), AllGather the
   pooled activations between levels (z-padded gather buffers so per-core
   reads are a single dynamic-offset DMA). Levels 3-6 (grids 8/4/2/1) are
   replicated on every core.
 - Convs are bf16 matmuls (fp32 PSUM accumulation): activations channel-major
   [C, z, y, x] in SBUF (y/x zero-padded), 27 shifted-window matmuls
   accumulated in PSUM.
 - conv1 of block 0 uses a host-side im2col (81 contract rows, masked
   columns so the submanifold mask is free).
 - 64-channel contractions (L0 conv2, L1 conv1) pack z-pairs into K=128 via
   duplicated storage; L0 conv2 additionally pairs two output slices into
   the two 64-column halves of the PE array.
 - Submanifold masking: conv1 evictions multiply by a broadcast mask (also
   zeroes the out-of-grid halo slices); conv2 evictions add (mask-1)*BIG so
   the following max-pool ignores inactive voxels; pool result is multiplied
   by the pooled mask.
"""

import sys

sys.path.insert(0, "/opt/trn_rl_repo")

import numpy as np
import ml_dtypes
import concourse.bass as bass
import concourse.tile as tile
from concourse.tile import add_dep_helper
from concourse import bacc, mybir
from concourse.bass_utils import run_bass_kernel_spmd

NC = 8
GRID = 64
BIG = 1.0e30
CHANNELS = [(3, 64), (64, 64), (64, 96), (96, 96), (96, 128), (128, 128),
            (128, 160), (160, 160), (160, 192), (192, 192), (192, 224),
            (224, 224), (224, 256), (256, 256)]
F32 = mybir.dt.float32
BF16 = mybir.dt.bfloat16
NPBF16 = ml_dtypes.bfloat16

OFFSETS = [(dz, dy, dx) for dz in (-1, 0, 1) for dy in (-1, 0, 1) for dx in (-1, 0, 1)]
# 9 (dy,dx) pairs for z-pair-packed layers
DYDX = [(dy, dx) for dy in (-1, 0, 1) for dx in (-1, 0, 1)]


def _bf(x):
    return np.asarray(x, np.float32).astype(NPBF16)


def _ceil_div(a, b):
    return (a + b - 1) // b


def build_host_inputs(features, coors, Ws):
    """All host-side data marshalling. Returns (in_maps, meta)."""
    z, y, x = coors[:, 0], coors[:, 1], coors[:, 2]
    dense = np.zeros((GRID, GRID, GRID, 3), np.float32)
    mask0 = np.zeros((GRID, GRID, GRID), np.float32)
    dense[z, y, x] = features  # last write wins (matches XLA CPU scatter)
    mask0[z, y, x] = 1.0

    # mask pyramid
    masks = [mask0]
    m = mask0
    for _ in range(6):
        mr = m.reshape(m.shape[0] // 2, 2, m.shape[1] // 2, 2, m.shape[2] // 2, 2)
        m = mr.max(axis=(1, 3, 5))
        masks.append(m)

    # ---- X1col: host im2col for conv1 of block 0, column-masked ----
    # padded dense [3, 66, 66, 66]
    dpad = np.zeros((3, GRID + 2, GRID + 2, GRID + 2), np.float32)
    dpad[:, 1:-1, 1:-1, 1:-1] = dense.transpose(3, 0, 1, 2)
    # X1col_full[(off*3+ci), zglob, y, x] ; z in [-1, 65) handled per-core
    # build per-core slabs directly: core k conv1-out slices global [8k-1, 8k+9)
    x1cols = []
    for k in range(NC):
        xc = np.zeros((10, 81, GRID * GRID), np.float32)
        for sl in range(10):
            zg = 8 * k - 1 + sl
            if zg < 0 or zg >= GRID:
                continue
            cols = np.zeros((81, GRID, GRID), np.float32)
            for o, (dz, dy, dx) in enumerate(OFFSETS):
                # padded coords: (zg+dz+1, y+dy+1, x+dx+1) over y,x in [0,64)
                cols[o * 3:(o + 1) * 3] = dpad[:, zg + dz + 1,
                                               1 + dy:GRID + 1 + dy,
                                               1 + dx:GRID + 1 + dx]
            cols *= mask0[zg][None, :, :]
            xc[sl] = cols.reshape(81, -1)
        x1cols.append(_bf(xc))

    # ---- weight packs (all bf16) ----
    # W0 for im2col conv1: [81, 128] (co=64 duplicated for col-pairing)
    W0 = Ws[0]  # [3,3,3,3,64]
    w1col = np.zeros((81, 128), np.float32)
    for o, (dz, dy, dx) in enumerate(OFFSETS):
        w1col[o * 3:(o + 1) * 3, 0:64] = W0[dz + 1, dy + 1, dx + 1]
        w1col[o * 3:(o + 1) * 3, 64:128] = W0[dz + 1, dy + 1, dx + 1]
    w1col = _bf(w1col)

    def pack_pair(W):  # [3,3,3,cin,co] -> pair [2*cin, 9, co] + left [cin, 9, co]
        cin, co = W.shape[3], W.shape[4]
        wp = np.zeros((2 * cin, 9, co), np.float32)
        wl = np.zeros((cin, 9, co), np.float32)
        for j, (dy, dx) in enumerate(DYDX):
            wp[0:cin, j] = W[0, dy + 1, dx + 1]      # dz=-1
            wp[cin:2 * cin, j] = W[1, dy + 1, dx + 1]  # dz=0
            wl[:, j] = W[2, dy + 1, dx + 1]          # dz=+1
        return wp, wl

    w0p, w0l = pack_pair(Ws[1])   # L0 conv2 64->64
    w1p, w1l = pack_pair(Ws[2])   # L1 conv1 64->96
    w0l = np.concatenate([w0l, w0l], axis=0)  # [128, 9, 64] both halves
    w1l = np.concatenate([w1l, w1l], axis=0)  # [128, 9, 96]
    w0p, w0l, w1p, w1l = _bf(w0p), _bf(w0l), _bf(w1p), _bf(w1l)

    def pack_generic(W):  # -> list of [kchunk, 27, co] arrays
        cin, co = W.shape[3], W.shape[4]
        wf = W.reshape(27, cin, co)
        out = []
        for k0 in range(0, cin, 128):
            kc = min(128, cin - k0)
            out.append(_bf(np.ascontiguousarray(
                wf[:, k0:k0 + kc, :].transpose(1, 0, 2))))  # [kc, 27, co]
        return out

    gen_w = {}
    for li, wi in [("w1c2", 3), ("w2c1", 4), ("w2c2", 5), ("w3c1", 6),
                   ("w3c2", 7), ("w4c1", 8), ("w4c2", 9), ("w5c1", 10),
                   ("w5c2", 11)]:
        gen_w[li] = pack_generic(Ws[wi])
    # L6: center tap only (1^3 grid)
    for li, wi in [("w6c1", 12), ("w6c2", 13)]:
        W = Ws[wi]
        cin, co = W.shape[3], W.shape[4]
        wc = W[1, 1, 1]  # [cin, co]
        gen_w[li] = [_bf(np.ascontiguousarray(
            wc[k0:k0 + min(128, cin - k0)][:, None, :]))
            for k0 in range(0, cin, 128)]

    # ---- per-core mask arrays (fp32) ----
    # L0 maskneg for conv2-evict: [8, 4096]
    mn0 = [_bf((masks[0][8 * k:8 * k + 8] - 1.0) * BIG).reshape(8, -1)
           for k in range(NC)]
    # L0 pool-out multiply: m1 on core's L1 slices [4, 1024]
    m1p = [_bf(masks[1][4 * k:4 * k + 4]).reshape(4, -1)
           for k in range(NC)]

    def slab_mask(mask, z0, nsl):
        D2 = mask.shape[1] * mask.shape[2]
        out = np.zeros((nsl, D2), np.float32)
        for i in range(nsl):
            zg = z0 + i
            if 0 <= zg < mask.shape[0]:
                out[i] = mask[zg].reshape(-1)
        return out

    # L1 conv1-evict multiply mask (m1 x ingrid): slices [4k-1, 4k+5)
    m1mul = [_bf(slab_mask(masks[1], 4 * k - 1, 6)) for k in range(NC)]
    # L1 conv2-evict maskneg: slices [4k, 4k+4)
    mn1 = [_bf((slab_mask(masks[1], 4 * k, 4) - 1.0) * BIG)
           for k in range(NC)]
    # L1 pool-out multiply: m2 on core's L2 slices [2, 256]
    m2p = [_bf(slab_mask(masks[2], 2 * k, 2)) for k in range(NC)]
    # L2 conv1-evict multiply (m2 x ingrid): slices [2k-1, 2k+3)
    m2mul = [slab_mask(masks[2], 2 * k - 1, 4) for k in range(NC)]
    # L2 conv2-evict maskneg: slices [2k, 2k+2)
    mn2 = [((slab_mask(masks[2], 2 * k, 2) - 1.0) * BIG).astype(np.float32)
           for k in range(NC)]
    # L2 pool-out multiply: m3 on core's L3 slice [1, 64]
    m3p = [slab_mask(masks[3], k, 1) for k in range(NC)]
    # L3 (replicated): conv1-evict mul (m3 x ingrid) slices [-1, 9)
    m3mul_r = slab_mask(masks[3], -1, 10)
    mn3_r = ((slab_mask(masks[3], 0, 8) - 1.0) * BIG).astype(np.float32)
    m4p_r = slab_mask(masks[4], 0, 4)       # [4, 16]
    m4mul_r = slab_mask(masks[4], 0, 4)     # L4 out all valid (full grid)
    mn4_r = ((slab_mask(masks[4], 0, 4) - 1.0) * BIG).astype(np.float32)
    m5p_r = slab_mask(masks[5], 0, 2)
    m5mul_r = slab_mask(masks[5], 0, 2)
    mn5_r = ((slab_mask(masks[5], 0, 2) - 1.0) * BIG).astype(np.float32)
    m6p_r = slab_mask(masks[6], 0, 1)

    meta = {
        "mask_flags": {
            # whether the real mask (not just ingrid) has zeros at each level
            1: not np.all(masks[1] == 1.0),
            2: not np.all(masks[2] == 1.0),
            3: not np.all(masks[3] == 1.0),
            4: not np.all(masks[4] == 1.0),
            5: not np.all(masks[5] == 1.0),
            6: not np.all(masks[6] == 1.0),
        },
    }

    in_maps = []
    for k in range(NC):
        im = {
            "x1col": x1cols[k],
            "w1col": w1col,
            "w0p": w0p, "w1p": w1p,
            "w0l": w0l, "w1l": w1l,
            "mn0": mn0[k], "m1p": m1p[k],
            "m1mul": m1mul[k], "mn1": mn1[k], "m2p": m2p[k],
            "m2mul": m2mul[k], "mn2": mn2[k], "m3p": m3p[k],
            "m3mul": m3mul_r, "mn3": mn3_r, "m4p": m4p_r,
            "m4mul": m4mul_r, "mn4": mn4_r, "m5p": m5p_r,
            "m5mul": m5mul_r, "mn5": mn5_r, "m6p": m6p_r,
        }
        for name, chunks in gen_w.items():
            for ci, arr in enumerate(chunks):
                im[f"{name}_{ci}"] = arr
        in_maps.append(im)
    return in_maps, meta


def build_kernel(meta):
    import contextlib
    nc = bacc.Bacc("TRN2", target_bir_lowering=False, debug=False, num_devices=NC)
    mf = meta["mask_flags"]

    # ---------- DRAM I/O declarations ----------
    def din(name, shape, dt=BF16):
        return nc.dram_tensor(name, list(shape), dt, kind="ExternalInput")

    x1col = din("x1col", (10, 81, 4096))
    w1col_d = din("w1col", (81, 128))
    w0p_d = din("w0p", (128, 9, 64)); w0l_d = din("w0l", (128, 9, 64))
    w1p_d = din("w1p", (128, 9, 96)); w1l_d = din("w1l", (128, 9, 96))
    mn0_d = din("mn0", (8, 4096)); m1p_d = din("m1p", (4, 1024))
    m1mul_d = din("m1mul", (6, 1024)); mn1_d = din("mn1", (4, 1024))
    m2p_d = din("m2p", (2, 256))
    m2mul_d = din("m2mul", (4, 256), F32); mn2_d = din("mn2", (2, 256), F32)
    m3p_d = din("m3p", (1, 64), F32)
    m3mul_d = din("m3mul", (10, 64), F32); mn3_d = din("mn3", (8, 64), F32)
    m4p_d = din("m4p", (4, 16), F32); m4mul_d = din("m4mul", (4, 16), F32)
    mn4_d = din("mn4", (4, 16), F32)
    m5p_d = din("m5p", (2, 4), F32); m5mul_d = din("m5mul", (2, 4), F32)
    mn5_d = din("mn5", (2, 4), F32); m6p_d = din("m6p", (1, 1), F32)

    genw_d = {}
    genw_shapes = {
        "w1c2": [(96, 27, 96)], "w2c1": [(96, 27, 128)], "w2c2": [(128, 27, 128)],
        "w3c1": [(128, 27, 160)], "w3c2": [(128, 27, 160), (32, 27, 160)],
        "w4c1": [(128, 27, 192), (32, 27, 192)],
        "w4c2": [(128, 27, 192), (64, 27, 192)],
        "w5c1": [(128, 27, 224), (64, 27, 224)],
        "w5c2": [(128, 27, 224), (96, 27, 224)],
        "w6c1": [(128, 1, 256), (96, 1, 256)],
        "w6c2": [(128, 1, 256), (128, 1, 256)],
    }
    for name, shl in genw_shapes.items():
        genw_d[name] = [din(f"{name}_{i}", s) for i, s in enumerate(shl)]

    out_d = nc.dram_tensor("out", [1, 256], F32, kind="ExternalOutput")
    import os as _os
    DBG = bool(_os.environ.get("K_DEBUG"))
    dbg_d = {}
    if DBG:
        for nm, sh in [("dP0", (64, 4, 1156)), ("dA1", (128, 8, 1156)),
                       ("dB1", (96, 6, 1156)), ("dC1", (96, 4, 1024)),
                       ("dA2", (96, 6, 324)), ("dA3", (128, 12, 100)),
                       ("dB2", (128, 4, 324)), ("dC2", (128, 2, 256)),
                       ("dP2", (128, 1, 100)), ("dP4", (128, 216)),
                       ("dP5", (128, 64)), ("dP6", (128, 27))]:
            dbg_d[nm] = nc.dram_tensor(nm, list(sh), BF16, kind="ExternalOutput")

    with tile.TileContext(nc) as tc:
        ctx = contextlib.ExitStack()
        with ctx:
            pst = ctx.enter_context(tc.tile_pool(name="ps", bufs=6, space="PSUM"))
            drm = ctx.enter_context(tc.tile_pool(name="dram", bufs=1, space="DRAM"))
            glob = ctx.enter_context(tc.tile_pool(name="glob", bufs=1))

            pid = nc.sync.partition_id()

            _weng = [nc.gpsimd, nc.scalar]

            def wload(pool, d, shape=None, name=None, dt=BF16, eng=None):
                sh = shape or d.shape
                t = pool.tile(list(sh), dt, name=name or f"sb_{d.name}")
                if eng is None:
                    eng = _weng[0]
                    _weng.reverse()
                eng.dma_start(t[:], d[:])
                return t

            # zero tile for G-pad zeroing
            zt = glob.tile([128, 1156], BF16)
            nc.vector.memset(zt[:], 0.0)

            # DRAM gather buffers (Shared HBM: faster AllGather delivery).
            # Group-major layout: one gather tensor per pooled-z residue group
            # so each per-slice AllGather has a contiguous output and can fire
            # as soon as that slice's pool completes (overlapping compute).
            # G1g[g][1+i] = L1-input global slice 4i+g ; slots 0/9 zero pads.
            c1_d = drm.tile([4, 64, 1156], BF16)
            G1g = [nc.dram_tensor(f"G1g{g}", [10, 64, 1156], BF16,
                                  addr_space="Shared") for g in range(4)]
            # G2g[g][1+i] = L2-input global slice 2i+g ; pads 0,9,10.
            c2_d = drm.tile([2, 96, 324], BF16)
            G2g = [nc.dram_tensor(f"G2g{g}", [11, 96, 324], BF16,
                                  addr_space="Shared") for g in range(2)]
            c3_d = drm.tile([1, 128, 100], BF16)
            G3 = nc.dram_tensor("G3s", [12, 128, 100], BF16, addr_space="Shared")
            # spread DMA issue across engines: each issuing engine owns its
            # own DGE queue, and everything funneled through nc.sync was
            # serializing on a single queue at startup.
            gpad_insts = []
            for G in G1g:
                for s in (0, 9):
                    gpad_insts.append(nc.gpsimd.dma_start(G[s], zt[0:64, 0:1156]))
            for G in G2g:
                for s in (0, 9, 10):
                    gpad_insts.append(nc.gpsimd.dma_start(G[s], zt[0:96, 0:324]))
            for s in (0, 1, 10, 11):
                gpad_insts.append(nc.gpsimd.dma_start(G3[s], zt[0:128, 0:100]))

            # persistent tail tensors (small; cross level boundaries)
            P4a = glob.tile([128, 216], BF16); P4b = glob.tile([32, 216], BF16)
            P5a = glob.tile([128, 64], BF16); P5b = glob.tile([64, 64], BF16)
            P6a = glob.tile([128, 27], BF16); P6b = glob.tile([96, 27], BF16)
            X6a = glob.tile([128, 1], BF16); X6b = glob.tile([128, 1], BF16)
            outt = glob.tile([128, 2], F32)
            for t in (P4a, P4b, P5a, P5b, P6a, P6b):
                nc.vector.memset(t[:], 0.0)

            # preloaded broadcast masks for L2 + tail (off the critical path)
            def mload(d, n):
                t = glob.tile([128, n], F32, name=f"pm_{d.name}")
                nc.scalar.dma_start(t[:], d[:].flatten().unsqueeze(0)
                                    .to_broadcast((128, n)))
                return t
            m2mul_t = mload(m2mul_d, 1024); mn2_t = mload(mn2_d, 512)
            m3p_t = mload(m3p_d, 64)
            m3mul_t = mload(m3mul_d, 640); mn3_t = mload(mn3_d, 512)
            m4p_t = mload(m4p_d, 64); m4mul_t = mload(m4mul_d, 64)
            mn4_t = mload(mn4_d, 64)
            m5p_t = mload(m5p_d, 8); m5mul_t = mload(m5mul_d, 8)
            mn5_t = mload(mn5_d, 8); m6p_t = mload(m6p_d, 1)

            # preload the tail weights once: per-level weight pools reuse
            # freed SBUF and stall each level start behind the previous
            # level's last reads. (w2*/w3c1 stay per-level: SBUF budget.)
            w4c1_t = [wload(glob, d) for d in genw_d["w4c1"]]
            w4c2_t = [wload(glob, d) for d in genw_d["w4c2"]]
            w5c1_t = [wload(glob, d) for d in genw_d["w5c1"]]
            w5c2_t = [wload(glob, d) for d in genw_d["w5c2"]]
            w6c1_t = [wload(glob, d) for d in genw_d["w6c1"]]
            w6c2_t = [wload(glob, d) for d in genw_d["w6c2"]]

            # ================ LEVEL 0 ================
            with tc.tile_pool(name="l0w", bufs=1) as wp, \
                 tc.tile_pool(name="l0p", bufs=1) as pp, \
                 tc.tile_pool(name="l0s", bufs=2) as ss, \
                 tc.tile_pool(name="l0m", bufs=4) as sm:
                w1col_t = wload(wp, w1col_d)
                w0p_t = wload(wp, w0p_d)
                w0l_t = wload(wp, w0l_d)

                A0 = pp.tile([128, 4, 4356], BF16)
                C0 = pp.tile([64, 2, 4096], BF16)
                P0 = pp.tile([64, 4, 1156], BF16)
                # border-only zeroing: conv1/pool evictions fill the interior
                for _s in range(4):
                    av = A0[:, _s, :].rearrange("p (a b) -> p a b", b=66)
                    nc.vector.memset(av[:, 0, :], 0.0)
                    nc.vector.memset(av[:, 65, :], 0.0)
                    nc.vector.memset(av[:, 1:65, 0], 0.0)
                    nc.vector.memset(av[:, 1:65, 65], 0.0)
                for _s in range(4):
                    pv = P0[:, _s, :].rearrange("p (a b) -> p a b", b=34)
                    nc.vector.memset(pv[:, 0, :], 0.0)
                    nc.vector.memset(pv[:, 33, :], 0.0)
                    nc.vector.memset(pv[:, 1:33, 0], 0.0)
                    nc.vector.memset(pv[:, 1:33, 33], 0.0)

                def l0_conv1(sl):
                    xs = ss.tile([81, 4096], BF16, tag="x1s")
                    nc.sync.dma_start(xs[:], x1col[sl])
                    for chunk in range(8):
                        ps = pst.tile([64, 512], F32, tag="ps")
                        nc.tensor.matmul(ps[:], w1col_t[:, 0:64],
                                         xs[:, chunk * 512:chunk * 512 + 512],
                                         start=True, stop=True)
                        r0, r1 = sl % 4, (sl - 1) % 4
                        yb = chunk * 8
                        src = ps[:].rearrange("p (a b) -> p a b", b=64)
                        d0 = A0[0:64, r0, :].rearrange("p (a b) -> p a b", b=66)
                        d1 = A0[64:128, r1, :].rearrange("p (a b) -> p a b", b=66)
                        nc.scalar.copy(d0[:, yb + 1:yb + 9, 1:65], src)
                        nc.gpsimd.tensor_copy(d1[:, yb + 1:yb + 9, 1:65],
                                              d0[:, yb + 1:yb + 9, 1:65])

                def l0_conv2(z):
                    # ring r: rows0 = h1[local r mod 4 writer], i.e.
                    # conv1(sl) wrote rows0@sl%4 and rows64@(sl-1)%4.
                    # out z needs h1 locals (z, z+1, z+2); out z+1 one more.
                    rA = z % 4         # rows0=h1[z], rows64=h1[z+1]
                    rB = (z + 1) % 4   # rows0=h1[z+1], rows64=h1[z+2]
                    rD = (z + 3) % 4   # rows0=h1[z+3]
                    for chunk in range(8):
                        yb = chunk * 8
                        psA = pst.tile([64, 512], F32, tag="ps")
                        psB = pst.tile([64, 512], F32, tag="ps")
                        wA = A0[:, rA, :].rearrange("p (a b) -> p a b", b=66)
                        wB = A0[:, rB, :].rearrange("p (a b) -> p a b", b=66)
                        wD = A0[:, rD, :].rearrange("p (a b) -> p a b", b=66)
                        for j, (dy, dx) in enumerate(DYDX):
                            first, last = (j == 0), (j == 8)
                            ys = slice(yb + 1 + dy, yb + 9 + dy)
                            xsl = slice(1 + dx, 65 + dx)
                            vA = psA[:].rearrange("p (a b) -> p a b", b=64)
                            vB = psB[:].rearrange("p (a b) -> p a b", b=64)
                            nc.tensor.matmul(vA, w0p_t[:, j, :],
                                             wA[:, ys, xsl],
                                             start=first, stop=False)
                            nc.tensor.matmul(vB, w0p_t[:, j, :],
                                             wB[:, ys, xsl],
                                             start=first, stop=False)
                            nc.tensor.matmul(vA, w0l_t[64:128, j, :],
                                             wB[64:128, ys, xsl],
                                             start=False, stop=last)
                            nc.tensor.matmul(vB, w0l_t[0:64, j, :],
                                             wD[0:64, ys, xsl],
                                             start=False, stop=last)
                        for ps_, zz, h in ((psA, z, 0), (psB, z + 1, 1)):
                            mt = sm.tile([64, 512], BF16, tag="mn0")
                            nc.scalar.dma_start(
                                mt[:], mn0_d[zz, yb * 64:yb * 64 + 512]
                                .unsqueeze(0).to_broadcast((64, 512)))
                            nc.vector.tensor_add(
                                C0[:, h, yb * 64:yb * 64 + 512], ps_[:], mt[:])

                def l0_pool(z):
                    zp = z // 2
                    nc.vector.tensor_max(C0[:, 0, :], C0[:, 0, :], C0[:, 1, :])
                    v = C0[:, 0, :].rearrange("p (a b) -> p a b", b=64)
                    t2 = ss.tile([64, 32, 64], BF16, tag="pool0b", bufs=1)
                    nc.vector.tensor_max(t2[:], v[:, 0::2, :], v[:, 1::2, :])
                    t3 = ss.tile([64, 32, 32], BF16, tag="pool0c", bufs=1)
                    nc.vector.tensor_max(t3[:], t2[:, :, 0::2], t2[:, :, 1::2])
                    mt = sm.tile([64, 1024], BF16, tag="m1p", bufs=2)
                    nc.scalar.dma_start(mt[:], m1p_d[zp].unsqueeze(0)
                                        .to_broadcast((64, 1024)))
                    dst = P0[:, zp, :].rearrange("p (a b) -> p a b", b=34)
                    nc.vector.tensor_mul(
                        dst[:, 1:33, 1:33], t3[:],
                        mt[:].rearrange("p (a b) -> p a b", b=32))

                # split AllGather: gather each pooled slice as soon as it is
                # ready so the collective overlaps the remaining L0 compute.
                ag1s = []
                for sl in range(10):
                    l0_conv1(sl)
                    if sl >= 3 and (sl - 3) % 2 == 0:
                        zz = sl - 3
                        l0_conv2(zz)
                        l0_pool(zz)
                        zp = zz // 2
                        nc.sync.dma_start(c1_d[zp], P0[:, zp, :])
                        ag = nc.gpsimd.collective_compute(
                            "AllGather", mybir.AluOpType.bypass,
                            replica_groups=[list(range(NC))],
                            ins=[c1_d[zp].opt()], outs=[G1g[zp][1:9].opt()])
                        for gi in gpad_insts:
                            add_dep_helper(ag.ins, gi.ins,
                                           reason="G pads zeroed before gathers")
                        ag1s.append(ag)

            # ================ LEVEL 1 ================
            with tc.tile_pool(name="l1w", bufs=1) as wp, \
                 tc.tile_pool(name="l1p", bufs=1) as pp, \
                 tc.tile_pool(name="l1s", bufs=2) as ss, \
                 tc.tile_pool(name="l1m", bufs=4) as sm:
                w1p_t = wload(wp, w1p_d)
                w1l_t = wload(wp, w1l_d)
                w1c2_t = wload(wp, genw_d["w1c2"][0])

                A1 = pp.tile([128, 8, 1156], BF16)
                B1 = pp.tile([96, 6, 1156], BF16)
                C1 = pp.tile([96, 4, 1024], BF16)
                P1 = pp.tile([96, 2, 324], BF16)
                nc.vector.memset(B1[:], 0.0)
                nc.vector.memset(P1[:], 0.0)
                # A1 rows0 slot j = x1 slice 4k-2+j (j=0..7); rows64 slot j =
                # x1 slice 4k-1+j (j=0..6). Global slice s lives in group
                # g=s%4 at slot s//4+1; per-slice DMAs depend only on their
                # group's gather, so they stream in as the gathers land.
                for j in range(8):
                    s_g, s_c = (j + 2) % 4, (j + 2) // 4
                    r = nc.sync.dma_start(
                        A1[0:64, j, :],
                        G1g[s_g][bass.ds(pid + s_c, 1)]
                        .rearrange("z c v -> c (z v)"))
                    add_dep_helper(r.ins, ag1s[s_g].ins,
                                   reason="gather before dynamic read")
                for j in range(7):
                    s_g, s_c = (j + 3) % 4, (j + 3) // 4
                    r = nc.sync.dma_start(
                        A1[64:128, j, :],
                        G1g[s_g][bass.ds(pid + s_c, 1)]
                        .rearrange("z c v -> c (z v)"))
                    add_dep_helper(r.ins, ag1s[s_g].ins,
                                   reason="gather before dynamic read")

                def l1_conv1(sl):
                    # A1 rows0 idx i = x1[4k-2+i]; rows64 idx i = x1[4k-1+i].
                    # out sl (global 4k-1+sl): pair = A1[:, sl] (dz=-1,0);
                    # leftover dz=+1 = rows64 idx sl+1 == rows0 idx sl+2.
                    mt = sm.tile([96, 1024], BF16, tag="m1mul")
                    nc.scalar.dma_start(mt[:], m1mul_d[sl].unsqueeze(0)
                                        .to_broadcast((96, 1024)))
                    pss = [pst.tile([96, 512], F32, tag="ps", name=f"ps_l1_{sl}_{_c}") for _c in range(2)]
                    wA = A1[:, sl, :].rearrange("p (a b) -> p a b", b=34)
                    wB = A1[64:128, sl + 1, :].rearrange("p (a b) -> p a b", b=34)
                    wC = A1[0:64, sl + 2, :].rearrange("p (a b) -> p a b", b=34)
                    for j, (dy, dx) in enumerate(DYDX):
                        xsl = slice(1 + dx, 33 + dx)
                        for chunk in range(2):
                            yb = chunk * 16
                            ys = slice(yb + 1 + dy, yb + 17 + dy)
                            nc.tensor.matmul(
                                pss[chunk][:].rearrange("p (a b) -> p a b", b=32),
                                w1p_t[:, j, :], wA[:, ys, xsl],
                                start=(j == 0), stop=False)
                        ys0 = slice(1 + dy, 17 + dy)
                        ys1 = slice(17 + dy, 33 + dy)
                        nc.tensor.matmul(
                            pss[0][:].rearrange("p (a b) -> p a b", b=32),
                            w1l_t[64:128, j, :], wB[:, ys0, xsl],
                            start=False, stop=(j == 8))
                        nc.tensor.matmul(
                            pss[1][:].rearrange("p (a b) -> p a b", b=32),
                            w1l_t[0:64, j, :], wC[:, ys1, xsl],
                            start=False, stop=(j == 8))
                    for chunk in range(2):
                        yb = chunk * 16
                        dst = B1[:, sl, :].rearrange("p (a b) -> p a b", b=34)
                        nc.vector.tensor_mul(
                            dst[:, yb + 1:yb + 17, 1:33],
                            pss[chunk][:].rearrange("p (a b) -> p a b", b=32),
                            mt[:, yb * 32:yb * 32 + 512].rearrange(
                                "p (a b) -> p a b", b=32))

                def l1_conv2(sl):
                    mt = sm.tile([96, 1024], BF16, tag="mn1")
                    nc.scalar.dma_start(mt[:], mn1_d[sl].unsqueeze(0)
                                        .to_broadcast((96, 1024)))
                    for chunk in range(2):
                        yb = chunk * 16
                        ps = pst.tile([96, 512], F32, tag="ps")
                        for o, (dz, dy, dx) in enumerate(OFFSETS):
                            w = B1[:, sl + 1 + dz, :].rearrange(
                                "p (a b) -> p a b", b=34)
                            nc.tensor.matmul(
                                ps[:].rearrange("p (a b) -> p a b", b=32),
                                w1c2_t[:, o, :],
                                w[:, yb + 1 + dy:yb + 17 + dy, 1 + dx:33 + dx],
                                start=(o == 0), stop=(o == 26))
                        nc.vector.tensor_add(C1[:, sl, yb * 32:yb * 32 + 512],
                                             ps[:],
                                             mt[:, yb * 32:yb * 32 + 512])

                def l1_pool(zz):
                    zp = zz // 2
                    nc.vector.tensor_max(C1[:, zz, :], C1[:, zz, :], C1[:, zz + 1, :])
                    v = C1[:, zz, :].rearrange("p (a b) -> p a b", b=32)
                    t2 = ss.tile([96, 16, 32], BF16, tag="pool1b")
                    nc.vector.tensor_max(t2[:], v[:, 0::2, :], v[:, 1::2, :])
                    t3 = ss.tile([96, 16, 16], BF16, tag="pool1c")
                    nc.vector.tensor_max(t3[:], t2[:, :, 0::2], t2[:, :, 1::2])
                    mt = sm.tile([96, 256], BF16, tag="m2p")
                    nc.scalar.dma_start(mt[:], m2p_d[zp].unsqueeze(0)
                                        .to_broadcast((96, 256)))
                    dst = P1[:, zp, :].rearrange("p (a b) -> p a b", b=18)
                    nc.vector.tensor_mul(
                        dst[:, 1:17, 1:17], t3[:],
                        mt[:].rearrange("p (a b) -> p a b", b=16))

                if DBG:
                    nc.sync.dma_start(dbg_d["dP0"][:].rearrange("c z v -> c (z v)"), P0[:].rearrange("c z v -> c (z v)"))
                    nc.sync.dma_start(dbg_d["dA1"][:].rearrange("c z v -> c (z v)"), A1[:].rearrange("c z v -> c (z v)"))
                ag2s = []
                for sl in range(6):
                    l1_conv1(sl)
                    if sl >= 2:
                        l1_conv2(sl - 2)
                        if sl >= 3 and (sl - 3) % 2 == 0:
                            l1_pool(sl - 3)
                            zp = (sl - 3) // 2
                            nc.sync.dma_start(c2_d[zp], P1[:, zp, :])
                            ag = nc.gpsimd.collective_compute(
                                "AllGather", mybir.AluOpType.bypass,
                                replica_groups=[list(range(NC))],
                                ins=[c2_d[zp].opt()], outs=[G2g[zp][1:9].opt()])
                            for gi in gpad_insts:
                                add_dep_helper(ag.ins, gi.ins,
                                               reason="G pads zeroed before gathers")
                            ag2s.append(ag)

            # ================ LEVEL 2 ================
            with tc.tile_pool(name="l2w", bufs=1) as wp, \
                 tc.tile_pool(name="l2p", bufs=1) as pp, \
                 tc.tile_pool(name="l2s", bufs=2) as ss, \
                 tc.tile_pool(name="l2m", bufs=4) as sm:
                w2c1_t = wload(wp, genw_d["w2c1"][0])
                w2c2_t = wload(wp, genw_d["w2c2"][0])
                A2 = pp.tile([96, 6, 324], BF16)
                B2 = pp.tile([128, 4, 324], BF16)
                C2 = pp.tile([128, 2, 256], BF16)
                P2 = pp.tile([128, 1, 100], BF16)
                nc.vector.memset(B2[:], 0.0)
                nc.vector.memset(P2[:], 0.0)
                # A2 slot j = pooled global slice 2k-2+j (j=0..5); slice s in
                # group g=s%2 at slot s//2+1.
                for j in range(6):
                    s_g, s_c = j % 2, j // 2
                    r = nc.sync.dma_start(
                        A2[:, j, :],
                        G2g[s_g][bass.ds(pid + s_c, 1)]
                        .rearrange("z c v -> c (z v)"))
                    add_dep_helper(r.ins, ag2s[s_g].ins,
                                   reason="gather before dynamic read")

                if DBG:
                    nc.sync.dma_start(dbg_d["dA2"][:].rearrange("c z v -> c (z v)"), A2[:].rearrange("c z v -> c (z v)"))
                for s0 in (0, 2):
                    ps = pst.tile([128, 512], F32, tag="ps")
                    for o, (dz, dy, dx) in enumerate(OFFSETS):
                        w = A2[:].rearrange("p z (a b) -> p z a b", b=18)
                        nc.tensor.matmul(
                            ps[:].rearrange("p (z a b) -> p z a b", z=2, a=16),
                            w2c1_t[:, o, :],
                            w[:, s0 + dz + 1:s0 + dz + 3,
                              1 + dy:17 + dy, 1 + dx:17 + dx],
                            start=(o == 0), stop=(o == 26))
                    dst = B2[:].rearrange("p z (a b) -> p z a b", b=18)
                    nc.vector.tensor_mul(
                        dst[:, s0:s0 + 2, 1:17, 1:17],
                        ps[:].rearrange("p (z a b) -> p z a b", z=2, a=16),
                        m2mul_t[:, s0 * 256:s0 * 256 + 512]
                        .rearrange("p (z a b) -> p z a b", z=2, a=16))

                ps = pst.tile([128, 512], F32, tag="ps")
                for o, (dz, dy, dx) in enumerate(OFFSETS):
                    w = B2[:].rearrange("p z (a b) -> p z a b", b=18)
                    nc.tensor.matmul(
                        ps[:].rearrange("p (z a b) -> p z a b", z=2, a=16),
                        w2c2_t[:, o, :],
                        w[:, dz + 1:dz + 3, 1 + dy:17 + dy, 1 + dx:17 + dx],
                        start=(o == 0), stop=(o == 26))
                if mf[2]:
                    nc.vector.tensor_add(C2[:].rearrange("p a b -> p (a b)"),
                                         ps[:], mn2_t[:])
                else:
                    nc.scalar.copy(C2[:].rearrange("p a b -> p (a b)"), ps[:])

                # L2 pool
                nc.vector.tensor_max(C2[:, 0, :], C2[:, 0, :], C2[:, 1, :])
                v = C2[:, 0, :].rearrange("p (a b) -> p a b", b=16)
                t2 = ss.tile([128, 8, 16], BF16, tag="pool2b")
                nc.vector.tensor_max(t2[:], v[:, 0::2, :], v[:, 1::2, :])
                dst = P2[:, 0, :].rearrange("p (a b) -> p a b", b=10)
                if mf[3]:
                    t3 = ss.tile([128, 8, 8], BF16, tag="pool2c")
                    nc.vector.tensor_max(t3[:], t2[:, :, 0::2], t2[:, :, 1::2])
                    nc.vector.tensor_mul(
                        dst[:, 1:9, 1:9], t3[:],
                        m3p_t[:].rearrange("p (a b) -> p a b", b=8))
                else:
                    nc.vector.tensor_max(dst[:, 1:9, 1:9],
                                         t2[:, :, 0::2], t2[:, :, 1::2])

                if DBG:
                    nc.sync.dma_start(dbg_d["dB2"][:].rearrange("c z v -> c (z v)"), B2[:].rearrange("c z v -> c (z v)"))
                    nc.sync.dma_start(dbg_d["dC2"][:].rearrange("c z v -> c (z v)"), C2[:].rearrange("c z v -> c (z v)"))
                    nc.sync.dma_start(dbg_d["dP2"][:].rearrange("c z v -> c (z v)"), P2[:].rearrange("c z v -> c (z v)"))
                nc.sync.dma_start(c3_d[:].rearrange("z c v -> c z v"), P2[:])

            # ---- AllGather L2 -> L3 ----
            ag3 = nc.gpsimd.collective_compute(
                "AllGather", mybir.AluOpType.bypass,
                replica_groups=[list(range(NC))],
                ins=[c3_d[:].opt()], outs=[G3[2:10].opt()])
            for gi in gpad_insts:
                add_dep_helper(ag3.ins, gi.ins, reason="G pads zeroed before gathers")

            # ================ LEVEL 3 (replicated) ================
            with tc.tile_pool(name="l3w", bufs=1) as wp, \
                 tc.tile_pool(name="l3p", bufs=1) as pp, \
                 tc.tile_pool(name="l3s", bufs=2) as ss, \
                 tc.tile_pool(name="l3m", bufs=4) as sm:
                w3c1_t = wload(wp, genw_d["w3c1"][0])
                w3c2_t = [wload(wp, d) for d in genw_d["w3c2"]]
                A3 = pp.tile([128, 12, 100], BF16)
                B3a = pp.tile([128, 10, 100], BF16)
                B3b = pp.tile([32, 10, 100], BF16)
                C3a = pp.tile([128, 512], BF16)
                C3b = pp.tile([32, 512], BF16)
                nc.vector.memset(B3a[:], 0.0)
                nc.vector.memset(B3b[:], 0.0)
                _r4 = nc.sync.dma_start(A3[:], G3[:].rearrange("z c v -> c z v"))
                add_dep_helper(_r4.ins, ag3.ins, reason="gather before read")

                if DBG:
                    nc.sync.dma_start(dbg_d["dA3"][:].rearrange("c z v -> c (z v)"), A3[:].rearrange("c z v -> c (z v)"))
                # conv1 (disjoint z-groups: B3 z 0..7 then 8..9)
                for (z0, nz) in ((0, 8), (8, 2)):
                    N = nz * 64
                    for (c0, co_n) in ((0, 128), (128, 32)):
                        ps = pst.tile([co_n, 512], F32, tag="ps")
                        for o, (dz, dy, dx) in enumerate(OFFSETS):
                            w = A3[:].rearrange("p z (a b) -> p z a b", b=10)
                            nc.tensor.matmul(
                                ps[:, 0:N].rearrange(
                                    "p (z a b) -> p z a b", z=nz, a=8),
                                w3c1_t[:, o, c0:c0 + co_n],
                                w[:, z0 + dz + 1:z0 + dz + 1 + nz,
                                  1 + dy:9 + dy, 1 + dx:9 + dx],
                                start=(o == 0), stop=(o == 26))
                        B3 = B3a if c0 == 0 else B3b
                        dst = B3[:].rearrange("p z (a b) -> p z a b", b=10)
                        nc.vector.tensor_mul(
                            dst[:, z0:z0 + nz, 1:9, 1:9],
                            ps[:, 0:N].rearrange(
                                "p (z a b) -> p z a b", z=nz, a=8),
                            m3mul_t[0:co_n, z0 * 64:z0 * 64 + N].rearrange(
                                "p (z a b) -> p z a b", z=nz, a=8))

                # conv2
                for (c0, co_n) in ((0, 128), (128, 32)):
                    ps = pst.tile([co_n, 512], F32, tag="ps")
                    for o, (dz, dy, dx) in enumerate(OFFSETS):
                        for ki, B3 in enumerate((B3a, B3b)):
                            w = B3[:].rearrange("p z (a b) -> p z a b", b=10)
                            nc.tensor.matmul(
                                ps[:].rearrange("p (z a b) -> p z a b",
                                                z=8, a=8),
                                w3c2_t[ki][:, o, c0:c0 + co_n],
                                w[:, dz + 1:dz + 9, 1 + dy:9 + dy,
                                  1 + dx:9 + dx],
                                start=(o == 0 and ki == 0),
                                stop=(o == 26 and ki == 1))
                    C3 = C3a if c0 == 0 else C3b
                    if mf[3]:
                        nc.vector.tensor_add(C3[:], ps[:], mn3_t[0:co_n, :])
                    else:
                        nc.scalar.copy(C3[:], ps[:])

                # pool -> P4
                for C3, P4, cn in ((C3a, P4a, 128), (C3b, P4b, 32)):
                    v = C3[:].rearrange("p (z v) -> p z v", v=64)
                    t1 = ss.tile([cn, 4, 64], BF16, tag="pool3a")
                    nc.vector.tensor_max(t1[:], v[:, 0::2, :], v[:, 1::2, :])
                    u = t1[:].rearrange("p z (a b) -> p z a b", b=8)
                    t2 = ss.tile([cn, 4, 4, 8], BF16, tag="pool3b")
                    nc.vector.tensor_max(t2[:], u[:, :, 0::2, :],
                                         u[:, :, 1::2, :])
                    dst = P4[:].rearrange("p (z a b) -> p z a b", z=6, a=6)
                    if mf[4]:
                        t3 = ss.tile([cn, 4, 4, 4], BF16, tag="pool3c")
                        nc.vector.tensor_max(t3[:], t2[:, :, :, 0::2],
                                             t2[:, :, :, 1::2])
                        nc.vector.tensor_mul(
                            dst[:, 1:5, 1:5, 1:5], t3[:],
                            m4p_t[0:cn, :].rearrange(
                                "p (z a b) -> p z a b", z=4, a=4))
                    else:
                        nc.vector.tensor_max(dst[:, 1:5, 1:5, 1:5],
                                             t2[:, :, :, 0::2],
                                             t2[:, :, :, 1::2])

            # ================ TAIL (levels 4-6, replicated) ================
            def tail_conv(wts, ins, outs, pg, og, mode, mtile):
                N = og * og * og
                noff = wts[0].shape[1]
                offs = OFFSETS if noff == 27 else [(0, 0, 0)]
                for (ot, c0, co_n, padded) in outs:
                    ps = pst.tile([co_n, max(N, 8)], F32, tag="ps")
                    nmm = len(offs) * len(ins)
                    i = 0
                    for o, (dz, dy, dx) in enumerate(offs):
                        for ki, it in enumerate(ins):
                            w = it[:].rearrange("p (z a b) -> p z a b",
                                                z=pg, a=pg)
                            nc.tensor.matmul(
                                ps[:, 0:N].rearrange(
                                    "p (z a b) -> p z a b", z=og, a=og),
                                wts[ki][:, o, c0:c0 + co_n],
                                w[:, 1 + dz:1 + dz + og, 1 + dy:1 + dy + og,
                                  1 + dx:1 + dx + og],
                                start=(i == 0), stop=(i == nmm - 1))
                            i += 1
                    if padded:
                        opg = og + 2
                        dst = ot[:].rearrange("p (z a b) -> p z a b",
                                              z=opg, a=opg)[:, 1:1 + og,
                                                            1:1 + og, 1:1 + og]
                    else:
                        dst = ot[:, 0:N].rearrange("p (z a b) -> p z a b",
                                                   z=og, a=og)
                    src = ps[:, 0:N].rearrange("p (z a b) -> p z a b",
                                               z=og, a=og)
                    if mode == "copy":
                        nc.scalar.copy(dst, src)
                    else:
                        mm = mtile[0:co_n, 0:N].rearrange(
                            "p (z a b) -> p z a b", z=og, a=og)
                        if mode == "mul":
                            nc.vector.tensor_mul(dst, src, mm)
                        else:
                            nc.vector.tensor_add(dst, src, mm)

            def tail_pool(ss, cs, ps_out, g, has_mask, mtile):
                go = g // 2
                for (ct, cn), (pt, _) in zip(cs, ps_out):
                    v = ct[:, 0:g * g * g].rearrange("p (z v) -> p z v",
                                                     v=g * g)
                    t1 = ss.tile([cn, go, g * g], BF16, tag=f"tp{g}a")
                    nc.vector.tensor_max(t1[:], v[:, 0::2, :], v[:, 1::2, :])
                    u = t1[:].rearrange("p z (a b) -> p z a b", b=g)
                    t2 = ss.tile([cn, go, go, g], BF16, tag=f"tp{g}b")
                    nc.vector.tensor_max(t2[:], u[:, :, 0::2, :],
                                         u[:, :, 1::2, :])
                    gp = go + 2
                    dst = pt[:].rearrange("p (z a b) -> p z a b", z=gp, a=gp)
                    if has_mask:
                        t3 = ss.tile([cn, go, go, go], BF16, tag=f"tp{g}c")
                        nc.vector.tensor_max(t3[:], t2[:, :, :, 0::2],
                                             t2[:, :, :, 1::2])
                        nc.vector.tensor_mul(
                            dst[:, 1:1 + go, 1:1 + go, 1:1 + go], t3[:],
                            mtile[0:cn, 0:go * go * go].rearrange(
                                "p (z a b) -> p z a b", z=go, a=go))
                    else:
                        nc.vector.tensor_max(
                            dst[:, 1:1 + go, 1:1 + go, 1:1 + go],
                            t2[:, :, :, 0::2], t2[:, :, :, 1::2])

            # ---- L4 ----
            with tc.tile_pool(name="l4w", bufs=1) as wp, \
                 tc.tile_pool(name="l4p", bufs=1) as pp, \
                 tc.tile_pool(name="l4s", bufs=2) as ss, \
                 tc.tile_pool(name="l4m", bufs=2) as sm:
                B4a = pp.tile([128, 216], BF16); B4b = pp.tile([64, 216], BF16)
                C4a = pp.tile([128, 64], BF16); C4b = pp.tile([64, 64], BF16)
                nc.vector.memset(B4a[:], 0.0)
                nc.vector.memset(B4b[:], 0.0)
                tail_conv(w4c1_t, [P4a, P4b],
                          [(B4a, 0, 128, True), (B4b, 128, 64, True)], 6, 4,
                          "mul" if mf[4] else "copy", m4mul_t)
                tail_conv(w4c2_t, [B4a, B4b],
                          [(C4a, 0, 128, False), (C4b, 128, 64, False)], 6, 4,
                          "add" if mf[4] else "copy", mn4_t)
                tail_pool(ss, [(C4a, 128), (C4b, 64)],
                          [(P5a, 128), (P5b, 64)], 4, mf[5], m5p_t)
                if DBG:
                    nc.sync.dma_start(dbg_d["dP4"][:], P4a[:])
                    nc.sync.dma_start(dbg_d["dP5"][:], P5a[:])

            # ---- L5 ----
            with tc.tile_pool(name="l5w", bufs=1) as wp, \
                 tc.tile_pool(name="l5p", bufs=1) as pp, \
                 tc.tile_pool(name="l5s", bufs=2) as ss, \
                 tc.tile_pool(name="l5m", bufs=2) as sm:
                B5a = pp.tile([128, 64], BF16); B5b = pp.tile([96, 64], BF16)
                C5a = pp.tile([128, 8], BF16); C5b = pp.tile([96, 8], BF16)
                nc.vector.memset(B5a[:], 0.0)
                nc.vector.memset(B5b[:], 0.0)
                tail_conv(w5c1_t, [P5a, P5b],
                          [(B5a, 0, 128, True), (B5b, 128, 96, True)], 4, 2,
                          "mul" if mf[5] else "copy", m5mul_t)
                tail_conv(w5c2_t, [B5a, B5b],
                          [(C5a, 0, 128, False), (C5b, 128, 96, False)], 4, 2,
                          "add" if mf[5] else "copy", mn5_t)
                tail_pool(ss, [(C5a, 128), (C5b, 96)],
                          [(P6a, 128), (P6b, 96)], 2, mf[6], m6p_t)
                if DBG:
                    nc.sync.dma_start(dbg_d["dP6"][:], P6a[:])

            # ---- L6 (1^3, center tap only) ----
            if True:
                for (ot, c0) in ((X6a, 0), (X6b, 128)):
                    ps = pst.tile([128, 8], F32, tag="ps")
                    nc.tensor.matmul(ps[:, 0:1], w6c1_t[0][:, 0, c0:c0 + 128],
                                     P6a[:, 13:14], start=True, stop=False)
                    nc.tensor.matmul(ps[:, 0:1], w6c1_t[1][:, 0, c0:c0 + 128],
                                     P6b[:, 13:14], start=False, stop=True)
                    nc.vector.tensor_copy(ot[:], ps[:, 0:1])
                for i, c0 in enumerate((0, 128)):
                    ps = pst.tile([128, 8], F32, tag="ps")
                    nc.tensor.matmul(ps[:, 0:1], w6c2_t[0][:, 0, c0:c0 + 128],
                                     X6a[:], start=True, stop=False)
                    nc.tensor.matmul(ps[:, 0:1], w6c2_t[1][:, 0, c0:c0 + 128],
                                     X6b[:], start=False, stop=True)
                    nc.scalar.copy(outt[:, i:i + 1], ps[:, 0:1])
            nc.sync.dma_start(out_d[0, 0:128], outt[:, 0])
            nc.sync.dma_start(out_d[0, 128:256], outt[:, 1])

    nc.compile()
    return nc



_CACHE = {}


def kernel(features, coors, W0, W1, W2, W3, W4, W5, W6, W7, W8, W9, W10, W11,
           W12, W13):
    features = np.asarray(features, np.float32)
    coors = np.asarray(coors, np.int32)
    Ws = [np.asarray(w, np.float32) for w in
          (W0, W1, W2, W3, W4, W5, W6, W7, W8, W9, W10, W11, W12, W13)]
    in_maps, meta = build_host_inputs(features, coors, Ws)
    key = tuple(sorted(meta["mask_flags"].items()))
    if key not in _CACHE:
        _CACHE[key] = build_kernel(meta)
    nc = _CACHE[key]
    res = run_bass_kernel_spmd(nc, in_maps, core_ids=list(range(NC)))
    out = res.results[0]["out"].reshape(256)
    return out.reshape(1, 1, 1, 1, 256).astype(np.float32)


if __name__ == "__main__":
    pass


# revision 45
# speedup vs baseline: 1.0434x; 1.0434x over previous
"""Trainium2 Bass kernel for the sparse submanifold 3D CNN (nn_Net_38963943309313).

Network: 7 blocks of 2 submanifold 3x3x3 convs on a 64^3 grid, 2x2x2 sparse
max-pools between blocks, channels 3->64->...->256, output [1,1,1,1,256].

Strategy (8 NeuronCores):
 - Shard z-slabs across cores for levels 0-2 (grids 64/32/16), AllGather the
   pooled activations between levels (z-padded gather buffers so per-core
   reads are a single dynamic-offset DMA). Levels 3-6 (grids 8/4/2/1) are
   replicated on every core.
 - Convs are bf16 matmuls (fp32 PSUM accumulation): activations channel-major
   [C, z, y, x] in SBUF (y/x zero-padded), 27 shifted-window matmuls
   accumulated in PSUM.
 - conv1 of block 0 uses a host-side im2col (81 contract rows, masked
   columns so the submanifold mask is free).
 - 64-channel contractions (L0 conv2, L1 conv1) pack z-pairs into K=128 via
   duplicated storage; L0 conv2 additionally pairs two output slices into
   the two 64-column halves of the PE array.
 - Submanifold masking: conv1 evictions multiply by a broadcast mask (also
   zeroes the out-of-grid halo slices); conv2 evictions add (mask-1)*BIG so
   the following max-pool ignores inactive voxels; pool result is multiplied
   by the pooled mask.
"""

import sys

sys.path.insert(0, "/opt/trn_rl_repo")

import numpy as np
import ml_dtypes
import concourse.bass as bass
import concourse.tile as tile
from concourse.tile import add_dep_helper
from concourse import bacc, mybir
from concourse.bass_utils import run_bass_kernel_spmd

NC = 8
GRID = 64
BIG = 1.0e30
CHANNELS = [(3, 64), (64, 64), (64, 96), (96, 96), (96, 128), (128, 128),
            (128, 160), (160, 160), (160, 192), (192, 192), (192, 224),
            (224, 224), (224, 256), (256, 256)]
F32 = mybir.dt.float32
BF16 = mybir.dt.bfloat16
NPBF16 = ml_dtypes.bfloat16

OFFSETS = [(dz, dy, dx) for dz in (-1, 0, 1) for dy in (-1, 0, 1) for dx in (-1, 0, 1)]
# 9 (dy,dx) pairs for z-pair-packed layers
DYDX = [(dy, dx) for dy in (-1, 0, 1) for dx in (-1, 0, 1)]


def _bf(x):
    return np.asarray(x, np.float32).astype(NPBF16)


def _ceil_div(a, b):
    return (a + b - 1) // b


def build_host_inputs(features, coors, Ws):
    """All host-side data marshalling. Returns (in_maps, meta)."""
    z, y, x = coors[:, 0], coors[:, 1], coors[:, 2]
    dense = np.zeros((GRID, GRID, GRID, 3), np.float32)
    mask0 = np.zeros((GRID, GRID, GRID), np.float32)
    dense[z, y, x] = features  # last write wins (matches XLA CPU scatter)
    mask0[z, y, x] = 1.0

    # mask pyramid
    masks = [mask0]
    m = mask0
    for _ in range(6):
        mr = m.reshape(m.shape[0] // 2, 2, m.shape[1] // 2, 2, m.shape[2] // 2, 2)
        m = mr.max(axis=(1, 3, 5))
        masks.append(m)

    # ---- X1col: host im2col for conv1 of block 0, column-masked ----
    # padded dense [3, 66, 66, 66]
    dpad = np.zeros((3, GRID + 2, GRID + 2, GRID + 2), np.float32)
    dpad[:, 1:-1, 1:-1, 1:-1] = dense.transpose(3, 0, 1, 2)
    # X1col_full[(off*3+ci), zglob, y, x] ; z in [-1, 65) handled per-core
    # build per-core slabs directly: core k conv1-out slices global [8k-1, 8k+9)
    x1cols = []
    for k in range(NC):
        xc = np.zeros((10, 81, GRID * GRID), np.float32)
        for sl in range(10):
            zg = 8 * k - 1 + sl
            if zg < 0 or zg >= GRID:
                continue
            cols = np.zeros((81, GRID, GRID), np.float32)
            for o, (dz, dy, dx) in enumerate(OFFSETS):
                # padded coords: (zg+dz+1, y+dy+1, x+dx+1) over y,x in [0,64)
                cols[o * 3:(o + 1) * 3] = dpad[:, zg + dz + 1,
                                               1 + dy:GRID + 1 + dy,
                                               1 + dx:GRID + 1 + dx]
            cols *= mask0[zg][None, :, :]
            xc[sl] = cols.reshape(81, -1)
        x1cols.append(_bf(xc))

    # ---- weight packs (all bf16) ----
    # W0 for im2col conv1: [81, 128] (co=64 duplicated for col-pairing)
    W0 = Ws[0]  # [3,3,3,3,64]
    w1col = np.zeros((81, 128), np.float32)
    for o, (dz, dy, dx) in enumerate(OFFSETS):
        w1col[o * 3:(o + 1) * 3, 0:64] = W0[dz + 1, dy + 1, dx + 1]
        w1col[o * 3:(o + 1) * 3, 64:128] = W0[dz + 1, dy + 1, dx + 1]
    w1col = _bf(w1col)

    def pack_pair(W):  # [3,3,3,cin,co] -> pair [2*cin, 9, co] + left [cin, 9, co]
        cin, co = W.shape[3], W.shape[4]
        wp = np.zeros((2 * cin, 9, co), np.float32)
        wl = np.zeros((cin, 9, co), np.float32)
        for j, (dy, dx) in enumerate(DYDX):
            wp[0:cin, j] = W[0, dy + 1, dx + 1]      # dz=-1
            wp[cin:2 * cin, j] = W[1, dy + 1, dx + 1]  # dz=0
            wl[:, j] = W[2, dy + 1, dx + 1]          # dz=+1
        return wp, wl

    w0p, w0l = pack_pair(Ws[1])   # L0 conv2 64->64
    w1p, w1l = pack_pair(Ws[2])   # L1 conv1 64->96
    w0l = np.concatenate([w0l, w0l], axis=0)  # [128, 9, 64] both halves
    w1l = np.concatenate([w1l, w1l], axis=0)  # [128, 9, 96]
    w0p, w0l, w1p, w1l = _bf(w0p), _bf(w0l), _bf(w1p), _bf(w1l)

    def pack_generic(W):  # -> list of [kchunk, 27, co] arrays
        cin, co = W.shape[3], W.shape[4]
        wf = W.reshape(27, cin, co)
        out = []
        for k0 in range(0, cin, 128):
            kc = min(128, cin - k0)
            out.append(_bf(np.ascontiguousarray(
                wf[:, k0:k0 + kc, :].transpose(1, 0, 2))))  # [kc, 27, co]
        return out

    gen_w = {}
    for li, wi in [("w1c2", 3), ("w2c1", 4), ("w2c2", 5), ("w3c1", 6),
                   ("w3c2", 7), ("w4c1", 8), ("w4c2", 9), ("w5c1", 10),
                   ("w5c2", 11)]:
        gen_w[li] = pack_generic(Ws[wi])
    # L6: center tap only (1^3 grid)
    for li, wi in [("w6c1", 12), ("w6c2", 13)]:
        W = Ws[wi]
        cin, co = W.shape[3], W.shape[4]
        wc = W[1, 1, 1]  # [cin, co]
        gen_w[li] = [_bf(np.ascontiguousarray(
            wc[k0:k0 + min(128, cin - k0)][:, None, :]))
            for k0 in range(0, cin, 128)]

    # ---- per-core mask arrays (fp32) ----
    # L0 maskneg for conv2-evict: [8, 4096]
    mn0 = [_bf((masks[0][8 * k:8 * k + 8] - 1.0) * BIG).reshape(8, -1)
           for k in range(NC)]
    # L0 pool-out multiply: m1 on core's L1 slices [4, 1024]
    m1p = [_bf(masks[1][4 * k:4 * k + 4]).reshape(4, -1)
           for k in range(NC)]

    def slab_mask(mask, z0, nsl):
        D2 = mask.shape[1] * mask.shape[2]
        out = np.zeros((nsl, D2), np.float32)
        for i in range(nsl):
            zg = z0 + i
            if 0 <= zg < mask.shape[0]:
                out[i] = mask[zg].reshape(-1)
        return out

    # L1 conv1-evict multiply mask (m1 x ingrid): slices [4k-1, 4k+5)
    m1mul = [_bf(slab_mask(masks[1], 4 * k - 1, 6)) for k in range(NC)]
    # L1 conv2-evict maskneg: slices [4k, 4k+4)
    mn1 = [_bf((slab_mask(masks[1], 4 * k, 4) - 1.0) * BIG)
           for k in range(NC)]
    # L1 pool-out multiply: m2 on core's L2 slices [2, 256]
    m2p = [_bf(slab_mask(masks[2], 2 * k, 2)) for k in range(NC)]
    # L2 conv1-evict multiply (m2 x ingrid): slices [2k-1, 2k+3)
    m2mul = [slab_mask(masks[2], 2 * k - 1, 4) for k in range(NC)]
    # L2 conv2-evict maskneg: slices [2k, 2k+2)
    mn2 = [((slab_mask(masks[2], 2 * k, 2) - 1.0) * BIG).astype(np.float32)
           for k in range(NC)]
    # L2 pool-out multiply: m3 on core's L3 slice [1, 64]
    m3p = [slab_mask(masks[3], k, 1) for k in range(NC)]
    # L3 (replicated): conv1-evict mul (m3 x ingrid) slices [-1, 9)
    m3mul_r = slab_mask(masks[3], -1, 10)
    mn3_r = ((slab_mask(masks[3], 0, 8) - 1.0) * BIG).astype(np.float32)
    m4p_r = slab_mask(masks[4], 0, 4)       # [4, 16]
    m4mul_r = slab_mask(masks[4], 0, 4)     # L4 out all valid (full grid)
    mn4_r = ((slab_mask(masks[4], 0, 4) - 1.0) * BIG).astype(np.float32)
    m5p_r = slab_mask(masks[5], 0, 2)
    m5mul_r = slab_mask(masks[5], 0, 2)
    mn5_r = ((slab_mask(masks[5], 0, 2) - 1.0) * BIG).astype(np.float32)
    m6p_r = slab_mask(masks[6], 0, 1)

    meta = {
        "mask_flags": {
            # whether the real mask (not just ingrid) has zeros at each level
            1: not np.all(masks[1] == 1.0),
            2: not np.all(masks[2] == 1.0),
            3: not np.all(masks[3] == 1.0),
            4: not np.all(masks[4] == 1.0),
            5: not np.all(masks[5] == 1.0),
            6: not np.all(masks[6] == 1.0),
        },
    }

    in_maps = []
    for k in range(NC):
        im = {
            "x1col": x1cols[k],
            "w1col": w1col,
            "w0p": w0p, "w1p": w1p,
            "w0l": w0l, "w1l": w1l,
            "mn0": mn0[k], "m1p": m1p[k],
            "m1mul": m1mul[k], "mn1": mn1[k], "m2p": m2p[k],
            "m2mul": m2mul[k], "mn2": mn2[k], "m3p": m3p[k],
            "m3mul": m3mul_r, "mn3": mn3_r, "m4p": m4p_r,
            "m4mul": m4mul_r, "mn4": mn4_r, "m5p": m5p_r,
            "m5mul": m5mul_r, "mn5": mn5_r, "m6p": m6p_r,
        }
        for name, chunks in gen_w.items():
            for ci, arr in enumerate(chunks):
                im[f"{name}_{ci}"] = arr
        in_maps.append(im)
    return in_maps, meta


def build_kernel(meta):
    import contextlib
    nc = bacc.Bacc("TRN2", target_bir_lowering=False, debug=False, num_devices=NC)
    mf = meta["mask_flags"]

    # ---------- DRAM I/O declarations ----------
    def din(name, shape, dt=BF16):
        return nc.dram_tensor(name, list(shape), dt, kind="ExternalInput")

    x1col = din("x1col", (10, 81, 4096))
    w1col_d = din("w1col", (81, 128))
    w0p_d = din("w0p", (128, 9, 64)); w0l_d = din("w0l", (128, 9, 64))
    w1p_d = din("w1p", (128, 9, 96)); w1l_d = din("w1l", (128, 9, 96))
    mn0_d = din("mn0", (8, 4096)); m1p_d = din("m1p", (4, 1024))
    m1mul_d = din("m1mul", (6, 1024)); mn1_d = din("mn1", (4, 1024))
    m2p_d = din("m2p", (2, 256))
    m2mul_d = din("m2mul", (4, 256), F32); mn2_d = din("mn2", (2, 256), F32)
    m3p_d = din("m3p", (1, 64), F32)
    m3mul_d = din("m3mul", (10, 64), F32); mn3_d = din("mn3", (8, 64), F32)
    m4p_d = din("m4p", (4, 16), F32); m4mul_d = din("m4mul", (4, 16), F32)
    mn4_d = din("mn4", (4, 16), F32)
    m5p_d = din("m5p", (2, 4), F32); m5mul_d = din("m5mul", (2, 4), F32)
    mn5_d = din("mn5", (2, 4), F32); m6p_d = din("m6p", (1, 1), F32)

    genw_d = {}
    genw_shapes = {
        "w1c2": [(96, 27, 96)], "w2c1": [(96, 27, 128)], "w2c2": [(128, 27, 128)],
        "w3c1": [(128, 27, 160)], "w3c2": [(128, 27, 160), (32, 27, 160)],
        "w4c1": [(128, 27, 192), (32, 27, 192)],
        "w4c2": [(128, 27, 192), (64, 27, 192)],
        "w5c1": [(128, 27, 224), (64, 27, 224)],
        "w5c2": [(128, 27, 224), (96, 27, 224)],
        "w6c1": [(128, 1, 256), (96, 1, 256)],
        "w6c2": [(128, 1, 256), (128, 1, 256)],
    }
    for name, shl in genw_shapes.items():
        genw_d[name] = [din(f"{name}_{i}", s) for i, s in enumerate(shl)]

    out_d = nc.dram_tensor("out", [1, 256], F32, kind="ExternalOutput")
    import os as _os
    DBG = bool(_os.environ.get("K_DEBUG"))
    dbg_d = {}
    if DBG:
        for nm, sh in [("dP0", (64, 4, 1156)), ("dA1", (128, 8, 1156)),
                       ("dB1", (96, 6, 1156)), ("dC1", (96, 4, 1024)),
                       ("dA2", (96, 6, 324)), ("dA3", (128, 12, 100)),
                       ("dB2", (128, 4, 324)), ("dC2", (128, 2, 256)),
                       ("dP2", (128, 1, 100)), ("dP4", (128, 216)),
                       ("dP5", (128, 64)), ("dP6", (128, 27))]:
            dbg_d[nm] = nc.dram_tensor(nm, list(sh), BF16, kind="ExternalOutput")

    with tile.TileContext(nc) as tc:
        ctx = contextlib.ExitStack()
        with ctx:
            pst = ctx.enter_context(tc.tile_pool(name="ps", bufs=8, space="PSUM"))
            drm = ctx.enter_context(tc.tile_pool(name="dram", bufs=1, space="DRAM"))
            glob = ctx.enter_context(tc.tile_pool(name="glob", bufs=1))

            pid = nc.sync.partition_id()

            _weng = [nc.gpsimd, nc.scalar]

            def wload(pool, d, shape=None, name=None, dt=BF16, eng=None):
                sh = shape or d.shape
                t = pool.tile(list(sh), dt, name=name or f"sb_{d.name}")
                if eng is None:
                    eng = _weng[0]
                    _weng.reverse()
                eng.dma_start(t[:], d[:])
                return t

            # zero tile for G-pad zeroing
            zt = glob.tile([128, 1156], BF16)
            nc.vector.memset(zt[:], 0.0)

            # DRAM gather buffers (Shared HBM: faster AllGather delivery).
            # Group-major layout: one gather tensor per pooled-z residue group
            # so each per-slice AllGather has a contiguous output and can fire
            # as soon as that slice's pool completes (overlapping compute).
            # G1g[g][1+i] = L1-input global slice 4i+g ; slots 0/9 zero pads.
            c1_d = drm.tile([4, 64, 1156], BF16)
            G1g = [nc.dram_tensor(f"G1g{g}", [10, 64, 1156], BF16,
                                  addr_space="Shared") for g in range(4)]
            # G2g[g][1+i] = L2-input global slice 2i+g ; pads 0,9,10.
            c2_d = drm.tile([2, 96, 324], BF16)
            G2g = [nc.dram_tensor(f"G2g{g}", [11, 96, 324], BF16,
                                  addr_space="Shared") for g in range(2)]
            c3_d = drm.tile([1, 128, 100], BF16)
            G3 = nc.dram_tensor("G3s", [12, 128, 100], BF16, addr_space="Shared")
            # spread DMA issue across engines: each issuing engine owns its
            # own DGE queue, and everything funneled through nc.sync was
            # serializing on a single queue at startup.
            gpad_insts = []
            for G in G1g:
                for s in (0, 9):
                    gpad_insts.append(nc.gpsimd.dma_start(G[s], zt[0:64, 0:1156]))
            for G in G2g:
                for s in (0, 9, 10):
                    gpad_insts.append(nc.gpsimd.dma_start(G[s], zt[0:96, 0:324]))
            for s in (0, 1, 10, 11):
                gpad_insts.append(nc.gpsimd.dma_start(G3[s], zt[0:128, 0:100]))

            # persistent tail tensors (small; cross level boundaries)
            P4a = glob.tile([128, 216], BF16); P4b = glob.tile([32, 216], BF16)
            P5a = glob.tile([128, 64], BF16); P5b = glob.tile([64, 64], BF16)
            P6a = glob.tile([128, 27], BF16); P6b = glob.tile([96, 27], BF16)
            X6a = glob.tile([128, 1], BF16); X6b = glob.tile([128, 1], BF16)
            outt = glob.tile([128, 2], F32)
            for t in (P4a, P4b, P5a, P5b, P6a, P6b):
                nc.vector.memset(t[:], 0.0)

            # preloaded broadcast masks for L2 + tail (off the critical path)
            def mload(d, n):
                t = glob.tile([128, n], F32, name=f"pm_{d.name}")
                nc.scalar.dma_start(t[:], d[:].flatten().unsqueeze(0)
                                    .to_broadcast((128, n)))
                return t
            m2mul_t = mload(m2mul_d, 1024); mn2_t = mload(mn2_d, 512)
            m3p_t = mload(m3p_d, 64)
            m3mul_t = mload(m3mul_d, 640); mn3_t = mload(mn3_d, 512)
            m4p_t = mload(m4p_d, 64); m4mul_t = mload(m4mul_d, 64)
            mn4_t = mload(mn4_d, 64)
            m5p_t = mload(m5p_d, 8); m5mul_t = mload(m5mul_d, 8)
            mn5_t = mload(mn5_d, 8); m6p_t = mload(m6p_d, 1)

            # preload the tail weights once: per-level weight pools reuse
            # freed SBUF and stall each level start behind the previous
            # level's last reads. (w2*/w3c1 stay per-level: SBUF budget.)
            w4c1_t = [wload(glob, d) for d in genw_d["w4c1"]]
            w4c2_t = [wload(glob, d) for d in genw_d["w4c2"]]
            w5c1_t = [wload(glob, d) for d in genw_d["w5c1"]]
            w5c2_t = [wload(glob, d) for d in genw_d["w5c2"]]
            w6c1_t = [wload(glob, d) for d in genw_d["w6c1"]]
            w6c2_t = [wload(glob, d) for d in genw_d["w6c2"]]

            # ================ LEVEL 0 ================
            with tc.tile_pool(name="l0w", bufs=1) as wp, \
                 tc.tile_pool(name="l0p", bufs=1) as pp, \
                 tc.tile_pool(name="l0s", bufs=2) as ss, \
                 tc.tile_pool(name="l0m", bufs=4) as sm:
                w1col_t = wload(wp, w1col_d)
                w0p_t = wload(wp, w0p_d)
                w0l_t = wload(wp, w0l_d)

                A0 = pp.tile([128, 4, 4356], BF16)
                C0 = pp.tile([64, 2, 4096], BF16)
                P0 = pp.tile([64, 4, 1156], BF16)
                # border-only zeroing: conv1/pool evictions fill the interior
                for _s in range(4):
                    av = A0[:, _s, :].rearrange("p (a b) -> p a b", b=66)
                    nc.vector.memset(av[:, 0, :], 0.0)
                    nc.vector.memset(av[:, 65, :], 0.0)
                    nc.vector.memset(av[:, 1:65, 0], 0.0)
                    nc.vector.memset(av[:, 1:65, 65], 0.0)
                for _s in range(4):
                    pv = P0[:, _s, :].rearrange("p (a b) -> p a b", b=34)
                    nc.vector.memset(pv[:, 0, :], 0.0)
                    nc.vector.memset(pv[:, 33, :], 0.0)
                    nc.vector.memset(pv[:, 1:33, 0], 0.0)
                    nc.vector.memset(pv[:, 1:33, 33], 0.0)

                def l0_conv1(sl):
                    xs = ss.tile([81, 4096], BF16, tag="x1s")
                    nc.sync.dma_start(xs[:], x1col[sl])
                    for chunk in range(8):
                        ps = pst.tile([64, 512], F32, tag="ps")
                        nc.tensor.matmul(ps[:], w1col_t[:, 0:64],
                                         xs[:, chunk * 512:chunk * 512 + 512],
                                         start=True, stop=True)
                        r0, r1 = sl % 4, (sl - 1) % 4
                        yb = chunk * 8
                        src = ps[:].rearrange("p (a b) -> p a b", b=64)
                        d0 = A0[0:64, r0, :].rearrange("p (a b) -> p a b", b=66)
                        d1 = A0[64:128, r1, :].rearrange("p (a b) -> p a b", b=66)
                        nc.scalar.copy(d0[:, yb + 1:yb + 9, 1:65], src)
                        nc.gpsimd.tensor_copy(d1[:, yb + 1:yb + 9, 1:65],
                                              d0[:, yb + 1:yb + 9, 1:65])

                def l0_conv2(z):
                    # ring r: rows0 = h1[local r mod 4 writer], i.e.
                    # conv1(sl) wrote rows0@sl%4 and rows64@(sl-1)%4.
                    # out z needs h1 locals (z, z+1, z+2); out z+1 one more.
                    rA = z % 4         # rows0=h1[z], rows64=h1[z+1]
                    rB = (z + 1) % 4   # rows0=h1[z+1], rows64=h1[z+2]
                    rD = (z + 3) % 4   # rows0=h1[z+3]
                    for chunk in range(8):
                        yb = chunk * 8
                        psA = pst.tile([64, 512], F32, tag="ps")
                        psB = pst.tile([64, 512], F32, tag="ps")
                        wA = A0[:, rA, :].rearrange("p (a b) -> p a b", b=66)
                        wB = A0[:, rB, :].rearrange("p (a b) -> p a b", b=66)
                        wD = A0[:, rD, :].rearrange("p (a b) -> p a b", b=66)
                        for j, (dy, dx) in enumerate(DYDX):
                            first, last = (j == 0), (j == 8)
                            ys = slice(yb + 1 + dy, yb + 9 + dy)
                            xsl = slice(1 + dx, 65 + dx)
                            vA = psA[:].rearrange("p (a b) -> p a b", b=64)
                            vB = psB[:].rearrange("p (a b) -> p a b", b=64)
                            nc.tensor.matmul(vA, w0p_t[:, j, :],
                                             wA[:, ys, xsl],
                                             start=first, stop=False)
                            nc.tensor.matmul(vB, w0p_t[:, j, :],
                                             wB[:, ys, xsl],
                                             start=first, stop=False)
                            nc.tensor.matmul(vA, w0l_t[64:128, j, :],
                                             wB[64:128, ys, xsl],
                                             start=False, stop=last)
                            nc.tensor.matmul(vB, w0l_t[0:64, j, :],
                                             wD[0:64, ys, xsl],
                                             start=False, stop=last)
                        for ps_, zz, h in ((psA, z, 0), (psB, z + 1, 1)):
                            mt = sm.tile([64, 512], BF16, tag="mn0")
                            nc.scalar.dma_start(
                                mt[:], mn0_d[zz, yb * 64:yb * 64 + 512]
                                .unsqueeze(0).to_broadcast((64, 512)))
                            nc.vector.tensor_add(
                                C0[:, h, yb * 64:yb * 64 + 512], ps_[:], mt[:])

                def l0_pool(z):
                    zp = z // 2
                    nc.vector.tensor_max(C0[:, 0, :], C0[:, 0, :], C0[:, 1, :])
                    v = C0[:, 0, :].rearrange("p (a b) -> p a b", b=64)
                    t2 = ss.tile([64, 32, 64], BF16, tag="pool0b", bufs=1)
                    nc.vector.tensor_max(t2[:], v[:, 0::2, :], v[:, 1::2, :])
                    t3 = ss.tile([64, 32, 32], BF16, tag="pool0c", bufs=1)
                    nc.vector.tensor_max(t3[:], t2[:, :, 0::2], t2[:, :, 1::2])
                    mt = sm.tile([64, 1024], BF16, tag="m1p", bufs=2)
                    nc.scalar.dma_start(mt[:], m1p_d[zp].unsqueeze(0)
                                        .to_broadcast((64, 1024)))
                    dst = P0[:, zp, :].rearrange("p (a b) -> p a b", b=34)
                    nc.vector.tensor_mul(
                        dst[:, 1:33, 1:33], t3[:],
                        mt[:].rearrange("p (a b) -> p a b", b=32))

                # split AllGather: gather each pooled slice as soon as it is
                # ready so the collective overlaps the remaining L0 compute.
                ag1s = []
                for sl in range(10):
                    l0_conv1(sl)
                    if sl >= 3 and (sl - 3) % 2 == 0:
                        zz = sl - 3
                        l0_conv2(zz)
                        l0_pool(zz)
                        zp = zz // 2
                        nc.sync.dma_start(c1_d[zp], P0[:, zp, :])
                        ag = nc.gpsimd.collective_compute(
                            "AllGather", mybir.AluOpType.bypass,
                            replica_groups=[list(range(NC))],
                            ins=[c1_d[zp].opt()], outs=[G1g[zp][1:9].opt()])
                        for gi in gpad_insts:
                            add_dep_helper(ag.ins, gi.ins,
                                           reason="G pads zeroed before gathers")
                        ag1s.append(ag)

            # ================ LEVEL 1 ================
            with tc.tile_pool(name="l1w", bufs=1) as wp, \
                 tc.tile_pool(name="l1p", bufs=1) as pp, \
                 tc.tile_pool(name="l1s", bufs=2) as ss, \
                 tc.tile_pool(name="l1m", bufs=4) as sm:
                w1p_t = wload(wp, w1p_d)
                w1l_t = wload(wp, w1l_d)
                w1c2_t = wload(wp, genw_d["w1c2"][0])

                A1 = pp.tile([128, 8, 1156], BF16)
                B1 = pp.tile([96, 6, 1156], BF16)
                C1 = pp.tile([96, 4, 1024], BF16)
                P1 = pp.tile([96, 2, 324], BF16)
                nc.vector.memset(B1[:], 0.0)
                nc.vector.memset(P1[:], 0.0)
                # A1 rows0 slot j = x1 slice 4k-2+j (j=0..7); rows64 slot j =
                # x1 slice 4k-1+j (j=0..6). Global slice s lives in group
                # g=s%4 at slot s//4+1; per-slice DMAs depend only on their
                # group's gather, so they stream in as the gathers land.
                for j in range(8):
                    s_g, s_c = (j + 2) % 4, (j + 2) // 4
                    r = nc.sync.dma_start(
                        A1[0:64, j, :],
                        G1g[s_g][bass.ds(pid + s_c, 1)]
                        .rearrange("z c v -> c (z v)"))
                    add_dep_helper(r.ins, ag1s[s_g].ins,
                                   reason="gather before dynamic read")
                for j in range(7):
                    s_g, s_c = (j + 3) % 4, (j + 3) // 4
                    r = nc.sync.dma_start(
                        A1[64:128, j, :],
                        G1g[s_g][bass.ds(pid + s_c, 1)]
                        .rearrange("z c v -> c (z v)"))
                    add_dep_helper(r.ins, ag1s[s_g].ins,
                                   reason="gather before dynamic read")

                def l1_conv1(sl):
                    # A1 rows0 idx i = x1[4k-2+i]; rows64 idx i = x1[4k-1+i].
                    # out sl (global 4k-1+sl): pair = A1[:, sl] (dz=-1,0);
                    # leftover dz=+1 = rows64 idx sl+1 == rows0 idx sl+2.
                    mt = sm.tile([96, 1024], BF16, tag="m1mul")
                    nc.scalar.dma_start(mt[:], m1mul_d[sl].unsqueeze(0)
                                        .to_broadcast((96, 1024)))
                    pss = [pst.tile([96, 512], F32, tag="ps", name=f"ps_l1_{sl}_{_c}") for _c in range(2)]
                    wA = A1[:, sl, :].rearrange("p (a b) -> p a b", b=34)
                    wB = A1[64:128, sl + 1, :].rearrange("p (a b) -> p a b", b=34)
                    wC = A1[0:64, sl + 2, :].rearrange("p (a b) -> p a b", b=34)
                    for j, (dy, dx) in enumerate(DYDX):
                        xsl = slice(1 + dx, 33 + dx)
                        for chunk in range(2):
                            yb = chunk * 16
                            ys = slice(yb + 1 + dy, yb + 17 + dy)
                            nc.tensor.matmul(
                                pss[chunk][:].rearrange("p (a b) -> p a b", b=32),
                                w1p_t[:, j, :], wA[:, ys, xsl],
                                start=(j == 0), stop=False)
                        ys0 = slice(1 + dy, 17 + dy)
                        ys1 = slice(17 + dy, 33 + dy)
                        nc.tensor.matmul(
                            pss[0][:].rearrange("p (a b) -> p a b", b=32),
                            w1l_t[64:128, j, :], wB[:, ys0, xsl],
                            start=False, stop=(j == 8))
                        nc.tensor.matmul(
                            pss[1][:].rearrange("p (a b) -> p a b", b=32),
                            w1l_t[0:64, j, :], wC[:, ys1, xsl],
                            start=False, stop=(j == 8))
                    for chunk in range(2):
                        yb = chunk * 16
                        dst = B1[:, sl, :].rearrange("p (a b) -> p a b", b=34)
                        nc.vector.tensor_mul(
                            dst[:, yb + 1:yb + 17, 1:33],
                            pss[chunk][:].rearrange("p (a b) -> p a b", b=32),
                            mt[:, yb * 32:yb * 32 + 512].rearrange(
                                "p (a b) -> p a b", b=32))

                def l1_conv2(sl):
                    mt = sm.tile([96, 1024], BF16, tag="mn1")
                    nc.scalar.dma_start(mt[:], mn1_d[sl].unsqueeze(0)
                                        .to_broadcast((96, 1024)))
                    for chunk in range(2):
                        yb = chunk * 16
                        ps = pst.tile([96, 512], F32, tag="ps")
                        for o, (dz, dy, dx) in enumerate(OFFSETS):
                            w = B1[:, sl + 1 + dz, :].rearrange(
                                "p (a b) -> p a b", b=34)
                            nc.tensor.matmul(
                                ps[:].rearrange("p (a b) -> p a b", b=32),
                                w1c2_t[:, o, :],
                                w[:, yb + 1 + dy:yb + 17 + dy, 1 + dx:33 + dx],
                                start=(o == 0), stop=(o == 26))
                        nc.vector.tensor_add(C1[:, sl, yb * 32:yb * 32 + 512],
                                             ps[:],
                                             mt[:, yb * 32:yb * 32 + 512])

                def l1_pool(zz):
                    zp = zz // 2
                    nc.vector.tensor_max(C1[:, zz, :], C1[:, zz, :], C1[:, zz + 1, :])
                    v = C1[:, zz, :].rearrange("p (a b) -> p a b", b=32)
                    t2 = ss.tile([96, 16, 32], BF16, tag="pool1b")
                    nc.vector.tensor_max(t2[:], v[:, 0::2, :], v[:, 1::2, :])
                    t3 = ss.tile([96, 16, 16], BF16, tag="pool1c")
                    nc.vector.tensor_max(t3[:], t2[:, :, 0::2], t2[:, :, 1::2])
                    mt = sm.tile([96, 256], BF16, tag="m2p")
                    nc.scalar.dma_start(mt[:], m2p_d[zp].unsqueeze(0)
                                        .to_broadcast((96, 256)))
                    dst = P1[:, zp, :].rearrange("p (a b) -> p a b", b=18)
                    nc.vector.tensor_mul(
                        dst[:, 1:17, 1:17], t3[:],
                        mt[:].rearrange("p (a b) -> p a b", b=16))

                if DBG:
                    nc.sync.dma_start(dbg_d["dP0"][:].rearrange("c z v -> c (z v)"), P0[:].rearrange("c z v -> c (z v)"))
                    nc.sync.dma_start(dbg_d["dA1"][:].rearrange("c z v -> c (z v)"), A1[:].rearrange("c z v -> c (z v)"))
                ag2s = []

                def l1_ag(zz):
                    zp = zz // 2
                    nc.sync.dma_start(c2_d[zp], P1[:, zp, :])
                    ag = nc.gpsimd.collective_compute(
                        "AllGather", mybir.AluOpType.bypass,
                        replica_groups=[list(range(NC))],
                        ins=[c2_d[zp].opt()], outs=[G2g[zp][1:9].opt()])
                    for gi in gpad_insts:
                        add_dep_helper(ag.ins, gi.ins,
                                       reason="G pads zeroed before gathers")
                    ag2s.append(ag)

                # conv1(2) first: it is the only conv1 whose input slices
                # avoid the last L0 gather group (residue 3), so L1 compute
                # starts before that final gather lands.
                l1_conv1(2)
                l1_conv1(0)
                l1_conv1(1)
                l1_conv2(0)
                l1_conv1(3)
                l1_conv2(1)
                l1_pool(0)
                l1_ag(0)
                l1_conv1(4)
                l1_conv2(2)
                l1_conv1(5)
                l1_conv2(3)
                l1_pool(2)
                l1_ag(2)

            # ================ LEVEL 2 ================
            with tc.tile_pool(name="l2w", bufs=1) as wp, \
                 tc.tile_pool(name="l2p", bufs=1) as pp, \
                 tc.tile_pool(name="l2s", bufs=2) as ss, \
                 tc.tile_pool(name="l2m", bufs=4) as sm:
                w2c1_t = wload(wp, genw_d["w2c1"][0])
                w2c2_t = wload(wp, genw_d["w2c2"][0])
                A2 = pp.tile([96, 6, 324], BF16)
                B2 = pp.tile([128, 4, 324], BF16)
                C2 = pp.tile([128, 2, 256], BF16)
                P2 = pp.tile([128, 1, 100], BF16)
                nc.vector.memset(B2[:], 0.0)
                nc.vector.memset(P2[:], 0.0)
                # A2 slot j = pooled global slice 2k-2+j (j=0..5); slice s in
                # group g=s%2 at slot s//2+1.
                for j in range(6):
                    s_g, s_c = j % 2, j // 2
                    r = nc.sync.dma_start(
                        A2[:, j, :],
                        G2g[s_g][bass.ds(pid + s_c, 1)]
                        .rearrange("z c v -> c (z v)"))
                    add_dep_helper(r.ins, ag2s[s_g].ins,
                                   reason="gather before dynamic read")

                if DBG:
                    nc.sync.dma_start(dbg_d["dA2"][:].rearrange("c z v -> c (z v)"), A2[:].rearrange("c z v -> c (z v)"))
                for s0 in (0, 2):
                    ps = pst.tile([128, 512], F32, tag="ps")
                    for o, (dz, dy, dx) in enumerate(OFFSETS):
                        w = A2[:].rearrange("p z (a b) -> p z a b", b=18)
                        nc.tensor.matmul(
                            ps[:].rearrange("p (z a b) -> p z a b", z=2, a=16),
                            w2c1_t[:, o, :],
                            w[:, s0 + dz + 1:s0 + dz + 3,
                              1 + dy:17 + dy, 1 + dx:17 + dx],
                            start=(o == 0), stop=(o == 26))
                    dst = B2[:].rearrange("p z (a b) -> p z a b", b=18)
                    nc.vector.tensor_mul(
                        dst[:, s0:s0 + 2, 1:17, 1:17],
                        ps[:].rearrange("p (z a b) -> p z a b", z=2, a=16),
                        m2mul_t[:, s0 * 256:s0 * 256 + 512]
                        .rearrange("p (z a b) -> p z a b", z=2, a=16))

                ps = pst.tile([128, 512], F32, tag="ps")
                for o, (dz, dy, dx) in enumerate(OFFSETS):
                    w = B2[:].rearrange("p z (a b) -> p z a b", b=18)
                    nc.tensor.matmul(
                        ps[:].rearrange("p (z a b) -> p z a b", z=2, a=16),
                        w2c2_t[:, o, :],
                        w[:, dz + 1:dz + 3, 1 + dy:17 + dy, 1 + dx:17 + dx],
                        start=(o == 0), stop=(o == 26))
                if mf[2]:
                    nc.vector.tensor_add(C2[:].rearrange("p a b -> p (a b)"),
                                         ps[:], mn2_t[:])
                else:
                    nc.scalar.copy(C2[:].rearrange("p a b -> p (a b)"), ps[:])

                # L2 pool
                nc.vector.tensor_max(C2[:, 0, :], C2[:, 0, :], C2[:, 1, :])
                v = C2[:, 0, :].rearrange("p (a b) -> p a b", b=16)
                t2 = ss.tile([128, 8, 16], BF16, tag="pool2b")
                nc.vector.tensor_max(t2[:], v[:, 0::2, :], v[:, 1::2, :])
                dst = P2[:, 0, :].rearrange("p (a b) -> p a b", b=10)
                if mf[3]:
                    t3 = ss.tile([128, 8, 8], BF16, tag="pool2c")
                    nc.vector.tensor_max(t3[:], t2[:, :, 0::2], t2[:, :, 1::2])
                    nc.vector.tensor_mul(
                        dst[:, 1:9, 1:9], t3[:],
                        m3p_t[:].rearrange("p (a b) -> p a b", b=8))
                else:
                    nc.vector.tensor_max(dst[:, 1:9, 1:9],
                                         t2[:, :, 0::2], t2[:, :, 1::2])

                if DBG:
                    nc.sync.dma_start(dbg_d["dB2"][:].rearrange("c z v -> c (z v)"), B2[:].rearrange("c z v -> c (z v)"))
                    nc.sync.dma_start(dbg_d["dC2"][:].rearrange("c z v -> c (z v)"), C2[:].rearrange("c z v -> c (z v)"))
                    nc.sync.dma_start(dbg_d["dP2"][:].rearrange("c z v -> c (z v)"), P2[:].rearrange("c z v -> c (z v)"))
                nc.sync.dma_start(c3_d[:].rearrange("z c v -> c z v"), P2[:])

            # ---- AllGather L2 -> L3 ----
            ag3 = nc.gpsimd.collective_compute(
                "AllGather", mybir.AluOpType.bypass,
                replica_groups=[list(range(NC))],
                ins=[c3_d[:].opt()], outs=[G3[2:10].opt()])
            for gi in gpad_insts:
                add_dep_helper(ag3.ins, gi.ins, reason="G pads zeroed before gathers")

            # ================ LEVEL 3 (replicated) ================
            with tc.tile_pool(name="l3w", bufs=1) as wp, \
                 tc.tile_pool(name="l3p", bufs=1) as pp, \
                 tc.tile_pool(name="l3s", bufs=2) as ss, \
                 tc.tile_pool(name="l3m", bufs=4) as sm:
                w3c1_t = wload(wp, genw_d["w3c1"][0])
                w3c2_t = [wload(wp, d) for d in genw_d["w3c2"]]
                A3 = pp.tile([128, 12, 100], BF16)
                B3a = pp.tile([128, 10, 100], BF16)
                B3b = pp.tile([32, 10, 100], BF16)
                C3a = pp.tile([128, 512], BF16)
                C3b = pp.tile([32, 512], BF16)
                nc.vector.memset(B3a[:], 0.0)
                nc.vector.memset(B3b[:], 0.0)
                _r4 = nc.sync.dma_start(A3[:], G3[:].rearrange("z c v -> c z v"))
                add_dep_helper(_r4.ins, ag3.ins, reason="gather before read")

                if DBG:
                    nc.sync.dma_start(dbg_d["dA3"][:].rearrange("c z v -> c (z v)"), A3[:].rearrange("c z v -> c (z v)"))
                # conv1 (disjoint z-groups: B3 z 0..7 then 8..9)
                for (z0, nz) in ((0, 8), (8, 2)):
                    N = nz * 64
                    for (c0, co_n) in ((0, 128), (128, 32)):
                        ps = pst.tile([co_n, 512], F32, tag="ps")
                        for o, (dz, dy, dx) in enumerate(OFFSETS):
                            w = A3[:].rearrange("p z (a b) -> p z a b", b=10)
                            nc.tensor.matmul(
                                ps[:, 0:N].rearrange(
                                    "p (z a b) -> p z a b", z=nz, a=8),
                                w3c1_t[:, o, c0:c0 + co_n],
                                w[:, z0 + dz + 1:z0 + dz + 1 + nz,
                                  1 + dy:9 + dy, 1 + dx:9 + dx],
                                start=(o == 0), stop=(o == 26))
                        B3 = B3a if c0 == 0 else B3b
                        dst = B3[:].rearrange("p z (a b) -> p z a b", b=10)
                        nc.vector.tensor_mul(
                            dst[:, z0:z0 + nz, 1:9, 1:9],
                            ps[:, 0:N].rearrange(
                                "p (z a b) -> p z a b", z=nz, a=8),
                            m3mul_t[0:co_n, z0 * 64:z0 * 64 + N].rearrange(
                                "p (z a b) -> p z a b", z=nz, a=8))

                # conv2
                for (c0, co_n) in ((0, 128), (128, 32)):
                    ps = pst.tile([co_n, 512], F32, tag="ps")
                    for o, (dz, dy, dx) in enumerate(OFFSETS):
                        for ki, B3 in enumerate((B3a, B3b)):
                            w = B3[:].rearrange("p z (a b) -> p z a b", b=10)
                            nc.tensor.matmul(
                                ps[:].rearrange("p (z a b) -> p z a b",
                                                z=8, a=8),
                                w3c2_t[ki][:, o, c0:c0 + co_n],
                                w[:, dz + 1:dz + 9, 1 + dy:9 + dy,
                                  1 + dx:9 + dx],
                                start=(o == 0 and ki == 0),
                                stop=(o == 26 and ki == 1))
                    C3 = C3a if c0 == 0 else C3b
                    if mf[3]:
                        nc.vector.tensor_add(C3[:], ps[:], mn3_t[0:co_n, :])
                    else:
                        nc.scalar.copy(C3[:], ps[:])

                # pool -> P4
                for C3, P4, cn in ((C3a, P4a, 128), (C3b, P4b, 32)):
                    v = C3[:].rearrange("p (z v) -> p z v", v=64)
                    t1 = ss.tile([cn, 4, 64], BF16, tag="pool3a")
                    nc.vector.tensor_max(t1[:], v[:, 0::2, :], v[:, 1::2, :])
                    u = t1[:].rearrange("p z (a b) -> p z a b", b=8)
                    t2 = ss.tile([cn, 4, 4, 8], BF16, tag="pool3b")
                    nc.vector.tensor_max(t2[:], u[:, :, 0::2, :],
                                         u[:, :, 1::2, :])
                    dst = P4[:].rearrange("p (z a b) -> p z a b", z=6, a=6)
                    if mf[4]:
                        t3 = ss.tile([cn, 4, 4, 4], BF16, tag="pool3c")
                        nc.vector.tensor_max(t3[:], t2[:, :, :, 0::2],
                                             t2[:, :, :, 1::2])
                        nc.vector.tensor_mul(
                            dst[:, 1:5, 1:5, 1:5], t3[:],
                            m4p_t[0:cn, :].rearrange(
                                "p (z a b) -> p z a b", z=4, a=4))
                    else:
                        nc.vector.tensor_max(dst[:, 1:5, 1:5, 1:5],
                                             t2[:, :, :, 0::2],
                                             t2[:, :, :, 1::2])

            # ================ TAIL (levels 4-6, replicated) ================
            def tail_conv(wts, ins, outs, pg, og, mode, mtile):
                N = og * og * og
                noff = wts[0].shape[1]
                offs = OFFSETS if noff == 27 else [(0, 0, 0)]
                for (ot, c0, co_n, padded) in outs:
                    ps = pst.tile([co_n, max(N, 8)], F32, tag="ps")
                    nmm = len(offs) * len(ins)
                    i = 0
                    for o, (dz, dy, dx) in enumerate(offs):
                        for ki, it in enumerate(ins):
                            w = it[:].rearrange("p (z a b) -> p z a b",
                                                z=pg, a=pg)
                            nc.tensor.matmul(
                                ps[:, 0:N].rearrange(
                                    "p (z a b) -> p z a b", z=og, a=og),
                                wts[ki][:, o, c0:c0 + co_n],
                                w[:, 1 + dz:1 + dz + og, 1 + dy:1 + dy + og,
                                  1 + dx:1 + dx + og],
                                start=(i == 0), stop=(i == nmm - 1))
                            i += 1
                    if padded:
                        opg = og + 2
                        dst = ot[:].rearrange("p (z a b) -> p z a b",
                                              z=opg, a=opg)[:, 1:1 + og,
                                                            1:1 + og, 1:1 + og]
                    else:
                        dst = ot[:, 0:N].rearrange("p (z a b) -> p z a b",
                                                   z=og, a=og)
                    src = ps[:, 0:N].rearrange("p (z a b) -> p z a b",
                                               z=og, a=og)
                    if mode == "copy":
                        nc.scalar.copy(dst, src)
                    else:
                        mm = mtile[0:co_n, 0:N].rearrange(
                            "p (z a b) -> p z a b", z=og, a=og)
                        if mode == "mul":
                            nc.vector.tensor_mul(dst, src, mm)
                        else:
                            nc.vector.tensor_add(dst, src, mm)

            def tail_pool(ss, cs, ps_out, g, has_mask, mtile):
                go = g // 2
                for (ct, cn), (pt, _) in zip(cs, ps_out):
                    v = ct[:, 0:g * g * g].rearrange("p (z v) -> p z v",
                                                     v=g * g)
                    t1 = ss.tile([cn, go, g * g], BF16, tag=f"tp{g}a")
                    nc.vector.tensor_max(t1[:], v[:, 0::2, :], v[:, 1::2, :])
                    u = t1[:].rearrange("p z (a b) -> p z a b", b=g)
                    t2 = ss.tile([cn, go, go, g], BF16, tag=f"tp{g}b")
                    nc.vector.tensor_max(t2[:], u[:, :, 0::2, :],
                                         u[:, :, 1::2, :])
                    gp = go + 2
                    dst = pt[:].rearrange("p (z a b) -> p z a b", z=gp, a=gp)
                    if has_mask:
                        t3 = ss.tile([cn, go, go, go], BF16, tag=f"tp{g}c")
                        nc.vector.tensor_max(t3[:], t2[:, :, :, 0::2],
                                             t2[:, :, :, 1::2])
                        nc.vector.tensor_mul(
                            dst[:, 1:1 + go, 1:1 + go, 1:1 + go], t3[:],
                            mtile[0:cn, 0:go * go * go].rearrange(
                                "p (z a b) -> p z a b", z=go, a=go))
                    else:
                        nc.vector.tensor_max(
                            dst[:, 1:1 + go, 1:1 + go, 1:1 + go],
                            t2[:, :, :, 0::2], t2[:, :, :, 1::2])

            # ---- L4 ----
            with tc.tile_pool(name="l4w", bufs=1) as wp, \
                 tc.tile_pool(name="l4p", bufs=1) as pp, \
                 tc.tile_pool(name="l4s", bufs=2) as ss, \
                 tc.tile_pool(name="l4m", bufs=2) as sm:
                B4a = pp.tile([128, 216], BF16); B4b = pp.tile([64, 216], BF16)
                C4a = pp.tile([128, 64], BF16); C4b = pp.tile([64, 64], BF16)
                nc.vector.memset(B4a[:], 0.0)
                nc.vector.memset(B4b[:], 0.0)
                tail_conv(w4c1_t, [P4a, P4b],
                          [(B4a, 0, 128, True), (B4b, 128, 64, True)], 6, 4,
                          "mul" if mf[4] else "copy", m4mul_t)
                tail_conv(w4c2_t, [B4a, B4b],
                          [(C4a, 0, 128, False), (C4b, 128, 64, False)], 6, 4,
                          "add" if mf[4] else "copy", mn4_t)
                tail_pool(ss, [(C4a, 128), (C4b, 64)],
                          [(P5a, 128), (P5b, 64)], 4, mf[5], m5p_t)
                if DBG:
                    nc.sync.dma_start(dbg_d["dP4"][:], P4a[:])
                    nc.sync.dma_start(dbg_d["dP5"][:], P5a[:])

            # ---- L5 ----
            with tc.tile_pool(name="l5w", bufs=1) as wp, \
                 tc.tile_pool(name="l5p", bufs=1) as pp, \
                 tc.tile_pool(name="l5s", bufs=2) as ss, \
                 tc.tile_pool(name="l5m", bufs=2) as sm:
                B5a = pp.tile([128, 64], BF16); B5b = pp.tile([96, 64], BF16)
                C5a = pp.tile([128, 8], BF16); C5b = pp.tile([96, 8], BF16)
                nc.vector.memset(B5a[:], 0.0)
                nc.vector.memset(B5b[:], 0.0)
                tail_conv(w5c1_t, [P5a, P5b],
                          [(B5a, 0, 128, True), (B5b, 128, 96, True)], 4, 2,
                          "mul" if mf[5] else "copy", m5mul_t)
                tail_conv(w5c2_t, [B5a, B5b],
                          [(C5a, 0, 128, False), (C5b, 128, 96, False)], 4, 2,
                          "add" if mf[5] else "copy", mn5_t)
                tail_pool(ss, [(C5a, 128), (C5b, 96)],
                          [(P6a, 128), (P6b, 96)], 2, mf[6], m6p_t)
                if DBG:
                    nc.sync.dma_start(dbg_d["dP6"][:], P6a[:])

            # ---- L6 (1^3, center tap only) ----
            if True:
                for (ot, c0) in ((X6a, 0), (X6b, 128)):
                    ps = pst.tile([128, 8], F32, tag="ps")
                    nc.tensor.matmul(ps[:, 0:1], w6c1_t[0][:, 0, c0:c0 + 128],
                                     P6a[:, 13:14], start=True, stop=False)
                    nc.tensor.matmul(ps[:, 0:1], w6c1_t[1][:, 0, c0:c0 + 128],
                                     P6b[:, 13:14], start=False, stop=True)
                    nc.vector.tensor_copy(ot[:], ps[:, 0:1])
                for i, c0 in enumerate((0, 128)):
                    ps = pst.tile([128, 8], F32, tag="ps")
                    nc.tensor.matmul(ps[:, 0:1], w6c2_t[0][:, 0, c0:c0 + 128],
                                     X6a[:], start=True, stop=False)
                    nc.tensor.matmul(ps[:, 0:1], w6c2_t[1][:, 0, c0:c0 + 128],
                                     X6b[:], start=False, stop=True)
                    nc.scalar.copy(outt[:, i:i + 1], ps[:, 0:1])
            nc.sync.dma_start(out_d[0, 0:128], outt[:, 0])
            nc.sync.dma_start(out_d[0, 128:256], outt[:, 1])

    nc.compile()
    return nc



_CACHE = {}


def kernel(features, coors, W0, W1, W2, W3, W4, W5, W6, W7, W8, W9, W10, W11,
           W12, W13):
    features = np.asarray(features, np.float32)
    coors = np.asarray(coors, np.int32)
    Ws = [np.asarray(w, np.float32) for w in
          (W0, W1, W2, W3, W4, W5, W6, W7, W8, W9, W10, W11, W12, W13)]
    in_maps, meta = build_host_inputs(features, coors, Ws)
    key = tuple(sorted(meta["mask_flags"].items()))
    if key not in _CACHE:
        _CACHE[key] = build_kernel(meta)
    nc = _CACHE[key]
    res = run_bass_kernel_spmd(nc, in_maps, core_ids=list(range(NC)))
    out = res.results[0]["out"].reshape(256)
    return out.reshape(1, 1, 1, 1, 256).astype(np.float32)


if __name__ == "__main__":
    pass
